# revision 1
# baseline (speedup 1.0000x reference)
"""Trainium2 Bass kernel for a 2-layer GCN encoder (GCNConv x2 + global mean pool).

Math: with A' = A + I and deg = indegree(A') (symmetric-norm GCN),
    gcn(h, W, b) = D^-1/2 A' D^-1/2 (h) W + b
factorized as  out = dinv * (A' @ (dinv * h)) @ W + b   (dinv = deg^-1/2)
so the SpMM is pure 0/1 structure; per-edge norms become per-node row scales.

Sharding: dst-node ranges across 8 cores (6272 padded rows each). Each core:
  - indirect-DMA gathers source rows of the (replicated) scaled feature table
    from HBM (one op per 128 rows; the HW honors one offset per partition),
  - streams each tile's self-loop block with a regular sequential DMA on the
    HWDGE queue (saves ~6% of gather descriptors, which are the bottleneck),
  - builds the 0/1 scatter matrix on VectorE (compare dst-offset vs iota;
    the self-loop chunk's column is an identity pattern),
  - scatter-adds via TensorE one-hot matmuls accumulating in PSUM (psum_T is
    [din, dstoff], already transposed for the dense W matmul),
  - dense W matmul + rank-1 bias matmul + fused relu/dinv-scale on ScalarE.
Two SPMD launches (layer 1 -> host allgather of the 1.6MB/core slabs ->
layer 2 + graph pooling via one-hot matmul over sorted batch ids).
"""
import math
import numpy as np
import ml_dtypes

from concourse import bass, mybir, tile, bacc
from concourse.bass_utils import run_bass_kernel_spmd
from concourse._compat import get_trn_type

N_CORES = 8
P = 128          # partitions / tile rows
D = 128          # feature dim
G = 512          # number of graphs (fixed by the problem)
F32 = mybir.dt.float32
BF16 = mybir.dt.bfloat16
I32 = mybir.dt.int32

USE_BF16 = True     # bf16 gather table (half the random-gather bytes; rel err ~1e-4)


# ---------------------------------------------------------------- host prep

def preprocess(x, edge_index, batch):
    N = x.shape[0]
    rows_per_core = int(math.ceil(N / (N_CORES * P))) * P
    npad = rows_per_core * N_CORES
    tiles_per_core = rows_per_core // P

    src = edge_index[0].astype(np.int64)
    dst = edge_index[1].astype(np.int64)
    # Self-loops are handled as a per-tile sequential-DMA chunk (no indirect
    # gather descriptors); only real edges go through the sorted stream.
    deg = (np.bincount(dst, minlength=N) + 1).astype(np.float32)
    dinv = 1.0 / np.sqrt(np.maximum(deg, 1.0))

    xhat = np.zeros((npad, D), dtype=np.float32)
    xhat[:N] = x.astype(np.float32) * dinv[:, None]

    order = np.argsort(dst)
    src_s = src[order].astype(np.int32)
    dst_s = dst[order]

    bounds = np.searchsorted(dst_s, np.arange(0, npad + 1, P))
    cnt = bounds[1:] - bounds[:-1]                      # per global tile
    chunks = (cnt + P - 1) // P                         # ceil
    chunks_2d = chunks.reshape(N_CORES, tiles_per_core)
    c_list = np.maximum(chunks_2d.max(axis=0), 1).astype(np.int64)  # per local tile
    sum_c = int(c_list.sum())
    c_max = int(c_list.max())

    # dstoff has one extra column per tile (the identity/self-loop chunk).
    sum_ca = sum_c + tiles_per_core
    src_idx = np.zeros((N_CORES, P, sum_c), dtype=np.int32)
    dstoff = np.full((N_CORES, P, sum_ca), float(P), dtype=np.float32)
    for k in range(N_CORES):
        col = 0
        cola = 0
        for t in range(tiles_per_core):
            C = int(c_list[t])
            gt = k * tiles_per_core + t
            s, e = int(bounds[gt]), int(bounds[gt + 1])
            n = e - s
            if n:
                bs = np.zeros(C * P, dtype=np.int32)
                bs[:n] = src_s[s:e]
                bo = np.full(C * P, float(P), dtype=np.float32)
                bo[:n] = (dst_s[s:e] - gt * P).astype(np.float32)
                src_idx[k, :, col:col + C] = bs.reshape(C, P).T
                dstoff[k, :, cola:cola + C] = bo.reshape(C, P).T
            # identity column: self-loop chunk scatters partition e -> column e
            # (pad rows contribute zero because xhat is zero there)
            dstoff[k, :, cola + C] = np.arange(P, dtype=np.float32)
            col += C
            cola += C + 1

    dinv_pad = np.zeros(npad, dtype=np.float32)
    dinv_pad[:N] = dinv
    dinv_slab = dinv_pad.reshape(N_CORES, tiles_per_core, P).transpose(0, 2, 1).copy()
    # sqrt(deg) per row (0 on pad rows): the bias is injected into PSUM as
    # sdeg[p]*b[j] so that the later *dinv row scale yields u*dinv + b exactly.
    sdeg_pad = np.zeros(npad, dtype=np.float32)
    sdeg_pad[:N] = np.sqrt(np.maximum(deg, 1.0))
    sdeg_rows = sdeg_pad.reshape(N_CORES, 1, tiles_per_core * P).copy()

    batch_pad = np.full(npad, -1, dtype=np.int64)
    batch_pad[:N] = batch.astype(np.int64)
    g0 = np.zeros(N_CORES, dtype=np.int64)
    batchoff = np.full((N_CORES, P, tiles_per_core), float(P), dtype=np.float32)
    for k in range(N_CORES):
        b = batch_pad[k * rows_per_core:(k + 1) * rows_per_core]
        real = b >= 0
        assert real.any()
        g0[k] = b[real].min()
        span = int(b[real].max() - g0[k]) + 1
        assert span <= P - 1, f"graph span {span} exceeds pooling tile"
        off = np.full(rows_per_core, float(P), dtype=np.float32)
        off[real] = (b[real] - g0[k]).astype(np.float32)
        batchoff[k] = off.reshape(tiles_per_core, P).T

    iota = np.tile(np.arange(P, dtype=np.float32), (P, c_max + 1))

    cnt_g = np.bincount(batch.astype(np.int64), minlength=G).astype(np.float32)

    return dict(N=N, npad=npad, rows_per_core=rows_per_core,
                tiles_per_core=tiles_per_core, c_list=c_list, sum_c=sum_c,
                sum_ca=sum_ca, c_max=c_max, src_idx=src_idx, dstoff=dstoff,
                dinv_slab=dinv_slab, sdeg_rows=sdeg_rows,
                batchoff=batchoff, g0=g0,
                iota=iota, xhat=xhat, cnt_g=cnt_g)


# ---------------------------------------------------------------- device

def build_layer(pre, last_layer: bool, reps: int = 1, bf16_table: bool = False):
    """One SPMD program: SpMM + dense matmul (+bias) per dst tile.
    last_layer=False: out = dinv*relu(dinv * z)  -> hhat slab [rows_per_core, D]
    last_layer=True:  h2 = dinv * z, pooled[goff] += sum h2 -> pooled [P, D]
    """
    tiles = pre['tiles_per_core']
    c_list = pre['c_list']
    sum_c = pre['sum_c']
    sum_ca = pre['sum_ca']
    c_max = pre['c_max']
    npad = pre['npad']

    TDT = BF16 if bf16_table else F32
    nc = bacc.Bacc(get_trn_type() or "TRN2", target_bir_lowering=False, debug=False)
    table = nc.dram_tensor("table", [npad, D], TDT, kind="ExternalInput").ap()
    self_rows = nc.dram_tensor("self_rows", [tiles * P, D], TDT, kind="ExternalInput").ap()
    src_idx = nc.dram_tensor("src_idx", [P, sum_c], I32, kind="ExternalInput").ap()
    dstoff = nc.dram_tensor("dstoff", [P, sum_ca], F32, kind="ExternalInput").ap()
    iota = nc.dram_tensor("iota", [P, (c_max + 1) * P], F32, kind="ExternalInput").ap()
    Wt = nc.dram_tensor("W", [D, D], F32, kind="ExternalInput").ap()
    bt = nc.dram_tensor("b", [1, D], F32, kind="ExternalInput").ap()
    dinv = nc.dram_tensor("dinv", [P, tiles], F32, kind="ExternalInput").ap()
    sdeg = nc.dram_tensor("sdeg", [1, tiles * P], F32, kind="ExternalInput").ap()
    if last_layer:
        batchoff = nc.dram_tensor("batchoff", [P, tiles], F32, kind="ExternalInput").ap()
        pooled = nc.dram_tensor("pooled", [P, D], F32, kind="ExternalOutput").ap()
    else:
        hhat = nc.dram_tensor("hhat", [tiles * P, D], TDT, kind="ExternalOutput").ap()

    with tile.TileContext(nc) as tc:
        with tc.tile_pool(name="const", bufs=1) as cp, \
             tc.tile_pool(name="gather", bufs=3) as gp, \
             tc.tile_pool(name="sel", bufs=3) as sp, \
             tc.tile_pool(name="small", bufs=3) as mp, \
             tc.tile_pool(name="ps1", bufs=2, space="PSUM") as pp1, \
             tc.tile_pool(name="ps2", bufs=2, space="PSUM") as pp2:
            src_t = cp.tile([P, sum_c], I32)
            dst_t = cp.tile([P, sum_ca], F32)
            iota_t = cp.tile([P, (c_max + 1) * P], F32)
            W_t = cp.tile([D, D], F32)
            b_t = cp.tile([1, D], F32)
            sdeg_t = cp.tile([1, tiles * P], F32)
            dinv_t = cp.tile([P, tiles], F32)
            nc.sync.dma_start(out=src_t[:], in_=src_idx[:])
            nc.sync.dma_start(out=dst_t[:], in_=dstoff[:])
            nc.sync.dma_start(out=iota_t[:], in_=iota[:])
            nc.sync.dma_start(out=W_t[:], in_=Wt[:])
            nc.sync.dma_start(out=b_t[:], in_=bt[:])
            nc.sync.dma_start(out=sdeg_t[:], in_=sdeg[:])
            nc.sync.dma_start(out=dinv_t[:], in_=dinv[:])
            if last_layer:
                boff_t = cp.tile([P, tiles], F32)
                nc.sync.dma_start(out=boff_t[:], in_=batchoff[:])
                pooled_sb = cp.tile([P, D], F32)

            for rep in range(reps):
                if last_layer:
                    nc.vector.memset(pooled_sb[:], 0.0)
                col = 0
                cola = 0
                for t in range(tiles):
                    C = int(c_list[t])
                    g = gp.tile([P, (c_max + 1) * D], TDT, tag="g")
                    # HW indirect DMA honors exactly one offset per partition
                    # per op, so gather chunk-by-chunk: 128 rows per op.
                    for c in range(C):
                        nc.gpsimd.indirect_dma_start(
                            out=g[:, c * D:(c + 1) * D], out_offset=None,
                            in_=table[:],
                            in_offset=bass.IndirectOffsetOnAxis(
                                ap=src_t[:, col + c:col + c + 1], axis=0))
                    # self-loop chunk: contiguous rows via the HWDGE queue
                    nc.sync.dma_start(out=g[:, C * D:(C + 1) * D],
                                      in_=self_rows[t * P:(t + 1) * P, :])
                    S = sp.tile([P, (c_max + 1) * D], TDT, tag="s")
                    nc.vector.tensor_tensor(
                        out=S[:, :(C + 1) * D].rearrange("p (c j) -> p c j", j=D),
                        in0=dst_t[:, cola:cola + C + 1].to_broadcast([P, C + 1, D]),
                        in1=iota_t[:, :(C + 1) * D].rearrange("p (c j) -> p c j", j=D),
                        op=mybir.AluOpType.is_equal)
                    psumT = pp1.tile([P, D], F32, space="PSUM", tag="pT")
                    for c in range(C + 1):
                        nc.tensor.matmul(out=psumT[:],
                                         lhsT=g[:, c * D:(c + 1) * D],
                                         rhs=S[:, c * D:(c + 1) * D],
                                         start=(c == 0), stop=(c == C))
                    lhs_sb = mp.tile([P, D], F32, tag="lhs")
                    nc.vector.tensor_copy(out=lhs_sb[:], in_=psumT[:])
                    psum2 = pp2.tile([P, D], F32, space="PSUM", tag="p2")
                    nc.tensor.matmul(out=psum2[:], lhsT=lhs_sb[:], rhs=W_t[:],
                                     start=True, stop=False)
                    nc.tensor.matmul(out=psum2[:], lhsT=sdeg_t[:, t * P:(t + 1) * P],
                                     rhs=b_t[:], start=False, stop=True)
                    out_sb = mp.tile([P, D], F32, tag="out")
                    if last_layer:
                        nc.scalar.activation(out=out_sb[:], in_=psum2[:],
                                             func=mybir.ActivationFunctionType.Copy,
                                             scale=dinv_t[:, t:t + 1])
                        Pt = sp.tile([P, D], F32, tag="pool_sel")
                        nc.vector.tensor_tensor(
                            out=Pt[:],
                            in0=boff_t[:, t:t + 1].to_broadcast([P, D]),
                            in1=iota_t[:, :D],
                            op=mybir.AluOpType.is_equal)
                        pool_ps = pp2.tile([P, D], F32, space="PSUM", tag="pool_ps")
                        nc.tensor.matmul(out=pool_ps[:], lhsT=Pt[:], rhs=out_sb[:],
                                         start=True, stop=True)
                        nc.vector.tensor_add(out=pooled_sb[:], in0=pooled_sb[:],
                                             in1=pool_ps[:])
                    else:
                        nc.scalar.activation(out=out_sb[:], in_=psum2[:],
                                             func=mybir.ActivationFunctionType.Relu,
                                             scale=dinv_t[:, t:t + 1])
                        hh_sb = mp.tile([P, D], TDT, tag="hh")
                        nc.vector.tensor_scalar_mul(out=hh_sb[:], in0=out_sb[:],
                                                    scalar1=dinv_t[:, t:t + 1])
                        nc.sync.dma_start(out=hhat[t * P:(t + 1) * P, :],
                                          in_=hh_sb[:])
                    col += C
                    cola += C + 1
                if last_layer:
                    nc.sync.dma_start(out=pooled[:], in_=pooled_sb[:])
    nc.compile()
    return nc


def _in_maps(pre, table_np, W, b, last_layer):
    maps = []
    rpc = pre['rows_per_core']
    for k in range(N_CORES):
        m = dict(table=table_np,
                 self_rows=np.ascontiguousarray(table_np[k * rpc:(k + 1) * rpc]),
                 src_idx=pre['src_idx'][k],
                 dstoff=pre['dstoff'][k],
                 iota=pre['iota'],
                 W=np.ascontiguousarray(W, dtype=np.float32),
                 b=np.ascontiguousarray(b, dtype=np.float32).reshape(1, D),
                 dinv=pre['dinv_slab'][k],
                 sdeg=pre['sdeg_rows'][k])
        if last_layer:
            m['batchoff'] = pre['batchoff'][k]
        maps.append(m)
    return maps


def kernel(x, edge_index, batch, W1, b1, W2, b2):
    x = np.asarray(x); edge_index = np.asarray(edge_index)
    batch = np.asarray(batch)
    W1 = np.asarray(W1); b1 = np.asarray(b1)
    W2 = np.asarray(W2); b2 = np.asarray(b2)

    pre = preprocess(x, edge_index, batch)
    core_ids = list(range(N_CORES))

    tdt = ml_dtypes.bfloat16 if USE_BF16 else np.float32
    table1 = pre['xhat'].astype(tdt)
    nc1 = build_layer(pre, last_layer=False, bf16_table=USE_BF16)
    res1 = run_bass_kernel_spmd(nc1, _in_maps(pre, table1, W1, b1, False),
                                core_ids).results

    h1hat = np.zeros((pre['npad'], D), dtype=tdt)
    rpc = pre['rows_per_core']
    for k in range(N_CORES):
        h1hat[k * rpc:(k + 1) * rpc] = res1[k]['hhat']

    nc2 = build_layer(pre, last_layer=True, bf16_table=USE_BF16)
    res2 = run_bass_kernel_spmd(nc2, _in_maps(pre, h1hat, W2, b2, True),
                                core_ids).results

    pooled = np.zeros((G, D), dtype=np.float32)
    for k in range(N_CORES):
        part = res2[k]['pooled']
        g0 = int(pre['g0'][k])
        span = min(P, G - g0)
        pooled[g0:g0 + span] += part[:span]
    return pooled / np.maximum(pre['cnt_g'], 1.0)[:, None]



# revision 14
# speedup vs baseline: 1.7736x; 1.7736x over previous
"""Trainium2 Bass kernel for a 2-layer GCN encoder (GCNConv x2 + global mean pool).

Math: with A' = A + I and deg = indegree(A') (symmetric-norm GCN),
    gcn(h, W, b) = D^-1/2 A' D^-1/2 (h) W + b
factorized as  out = dinv * (A' @ (dinv * h)) @ W + b   (dinv = deg^-1/2)
so the SpMM is pure 0/1 structure; per-edge norms become per-node row scales.

Sharding: dst-node ranges across 8 cores (6272 padded rows each). Each core:
  - bulk-gathers source rows of the (replicated) scaled feature table from HBM
    with InstDMAGatherAnt (one SWDGE op per ~7-tile group instead of one
    indirect op per 128 rows: the ~1us/op descriptor-generation fixed cost
    dominated the old kernel). Indices are int16, so the table is split at row
    32768 into lo/hi halves with one gather each per group,
  - streams each tile's self-loop block with one regular HWDGE DMA per group,
  - builds the 0/1 scatter matrix on VectorE in bf16 (compare dst-offset vs
    iota; the self-loop chunk's column is an identity pattern),
  - scatter-adds via TensorE one-hot matmuls accumulating in PSUM (psum_T is
    [din, dstoff], already transposed for the dense W matmul),
  - dense W matmul + rank-1 bias matmul; ScalarE applies relu and the row
    scale (layer 1 uses scale=dinv^2, folding both dinv factors around the
    relu: dinv*relu(dinv*zW + b) == relu(dinv^2*zW*... ) since dinv > 0).
Two SPMD launches (layer 1 -> host allgather of the 1.6MB/core slabs ->
layer 2 + graph pooling via one-hot matmul over sorted batch ids).
"""
import math
import numpy as np
import ml_dtypes

from concourse import bass, mybir, tile, bacc
from concourse.bass_utils import run_bass_kernel_spmd
from concourse._compat import get_trn_type

N_CORES = 8
P = 128          # partitions / tile rows
D = 128          # feature dim
G = 512          # number of graphs (fixed by the problem)
SPLIT = 32768    # dma_gather int16 index limit: table split row
GT = 7           # dst tiles per self-rows DMA group (49 = 7 x 7)
GATHER_BUDGET = 4096   # max idxs per dma_gather op (ring: 1024 descs/lane)
SINGLE_PACKET = False  # >64 descs/lane per op requires per-desc packets
F32 = mybir.dt.float32
BF16 = mybir.dt.bfloat16
I16 = mybir.dt.int16

USE_BF16 = True     # bf16 gather table (half the random-gather bytes)


def _make_groups(C, budget_slots):
    """Greedy consecutive-tile grouping: each group's slot count (sum C*P)
    stays <= budget (single tiles above budget get their own group)."""
    tiles = len(C)
    groups = []
    t = 0
    slot = 0
    while t < tiles:
        t2, n = t, 0
        while t2 < tiles and (t2 == t or n + int(C[t2]) * P <= budget_slots):
            n += int(C[t2]) * P
            t2 += 1
        groups.append((t, t2, slot, n))
        slot += n
        t = t2
    return groups


# ---------------------------------------------------------------- host prep

def preprocess(x, edge_index, batch):
    N = x.shape[0]
    rows_per_core = int(math.ceil(N / (N_CORES * P))) * P
    npad = rows_per_core * N_CORES
    tiles = rows_per_core // P
    n_tiles_g = N_CORES * tiles

    src = edge_index[0].astype(np.int64)
    dst = edge_index[1].astype(np.int64)
    # Self-loops are handled as a per-tile sequential-DMA chunk (no gather
    # slots); only real edges go through the sorted streams.
    deg = (np.bincount(dst, minlength=N) + 1).astype(np.float32)
    dinv = 1.0 / np.sqrt(np.maximum(deg, 1.0))

    xhat = np.zeros((npad, D), dtype=np.float32)
    xhat[:N] = x.astype(np.float32) * dinv[:, None]

    order = np.argsort(dst, kind="stable")
    src_s = src[order].astype(np.int64)
    dst_s = dst[order]
    bounds = np.searchsorted(dst_s, np.arange(0, npad + 1, P))

    # per-global-tile lo/hi edge partition (src < SPLIT goes to the lo gather)
    lo_srcs, lo_offs, hi_srcs, hi_offs = [], [], [], []
    n_lo = np.zeros(n_tiles_g, dtype=np.int64)
    n_hi = np.zeros(n_tiles_g, dtype=np.int64)
    for gt in range(n_tiles_g):
        s, e = int(bounds[gt]), int(bounds[gt + 1])
        seg_src = src_s[s:e]
        seg_off = (dst_s[s:e] - gt * P).astype(np.float32)
        m = seg_src < SPLIT
        lo_srcs.append(seg_src[m]); lo_offs.append(seg_off[m])
        hi_srcs.append(seg_src[~m] - SPLIT); hi_offs.append(seg_off[~m])
        n_lo[gt] = int(m.sum()); n_hi[gt] = int((~m).sum())

    C_lo = np.ceil(n_lo.reshape(N_CORES, tiles).max(axis=0) / P).astype(np.int64)
    C_hi = np.ceil(n_hi.reshape(N_CORES, tiles).max(axis=0) / P).astype(np.int64)
    c_tot = C_lo + C_hi                      # chunks per tile w/o self chunk
    c_max = int(c_tot.max())
    sum_ca = int(c_tot.sum()) + tiles        # dstoff cols incl self chunk

    L_lo = int(C_lo.sum()) * P               # lo gather slots per core
    L_hi = int(C_hi.sum()) * P
    lo_pos = np.concatenate([[0], np.cumsum(C_lo * P)])
    hi_pos = np.concatenate([[0], np.cumsum(C_hi * P)])

    idx_lo = np.zeros((N_CORES, L_lo), dtype=np.int32)
    idx_hi = np.zeros((N_CORES, L_hi), dtype=np.int32)
    dstoff = np.full((N_CORES, P, sum_ca), float(P), dtype=np.float32)
    for k in range(N_CORES):
        cola = 0
        for t in range(tiles):
            gt = k * tiles + t
            Cl, Ch = int(C_lo[t]), int(C_hi[t])
            nl, nh = int(n_lo[gt]), int(n_hi[gt])
            if nl:
                idx_lo[k, lo_pos[t]:lo_pos[t] + nl] = lo_srcs[gt]
                bo = np.full(Cl * P, float(P), dtype=np.float32)
                bo[:nl] = lo_offs[gt]
                dstoff[k, :, cola:cola + Cl] = bo.reshape(Cl, P).T
            cola += Cl
            if nh:
                idx_hi[k, hi_pos[t]:hi_pos[t] + nh] = hi_srcs[gt]
                bo = np.full(Ch * P, float(P), dtype=np.float32)
                bo[:nh] = hi_offs[gt]
                dstoff[k, :, cola:cola + Ch] = bo.reshape(Ch, P).T
            cola += Ch
            # identity column: self-loop chunk scatters partition e -> column e
            # (pad rows contribute zero because xhat is zero there)
            dstoff[k, :, cola] = np.arange(P, dtype=np.float32)
            cola += 1
    assert cola == sum_ca

    def idx_sbuf(a):
        # dma_gather idx layout: element i at partition i%16, col i//16,
        # replicated across all eight 16-partition stripes.
        sb = a.reshape(-1, 16).T
        return np.ascontiguousarray(np.tile(sb, (8, 1)).astype(np.int16))

    idx_lo_sb = np.stack([idx_sbuf(idx_lo[k]) for k in range(N_CORES)])
    idx_hi_sb = np.stack([idx_sbuf(idx_hi[k]) for k in range(N_CORES)])

    dinv_pad = np.zeros(npad, dtype=np.float32)
    dinv_pad[:N] = dinv
    dinv_slab = dinv_pad.reshape(N_CORES, tiles, P).transpose(0, 2, 1).copy()
    dinv2_slab = (dinv_slab * dinv_slab).copy()
    # sqrt(deg) per row (0 on pad rows): the bias is injected into PSUM as
    # sdeg[p]*b[j]; the later per-row scale turns it into the exact bias.
    sdeg_pad = np.zeros(npad, dtype=np.float32)
    sdeg_pad[:N] = np.sqrt(np.maximum(deg, 1.0))
    sdeg_rows = sdeg_pad.reshape(N_CORES, 1, tiles * P).copy()

    batch_pad = np.full(npad, -1, dtype=np.int64)
    batch_pad[:N] = batch.astype(np.int64)
    g0 = np.zeros(N_CORES, dtype=np.int64)
    batchoff = np.full((N_CORES, P, tiles), float(P), dtype=np.float32)
    for k in range(N_CORES):
        b = batch_pad[k * rows_per_core:(k + 1) * rows_per_core]
        real = b >= 0
        assert real.any()
        g0[k] = b[real].min()
        span = int(b[real].max() - g0[k]) + 1
        assert span <= P - 1, f"graph span {span} exceeds pooling tile"
        off = np.full(rows_per_core, float(P), dtype=np.float32)
        off[real] = (b[real] - g0[k]).astype(np.float32)
        batchoff[k] = off.reshape(tiles, P).T

    iota_bf = np.tile(np.arange(P, dtype=np.float32), (P, c_max + 1)
                      ).astype(ml_dtypes.bfloat16)
    iota_f32 = np.tile(np.arange(P, dtype=np.float32), (P, 1))
    cnt_g = np.bincount(batch.astype(np.int64), minlength=G).astype(np.float32)

    return dict(N=N, npad=npad, rows_per_core=rows_per_core,
                tiles_per_core=tiles, C_lo=C_lo, C_hi=C_hi, c_max=c_max,
                sum_ca=sum_ca, L_lo=L_lo, L_hi=L_hi,
                lo_pos=lo_pos, hi_pos=hi_pos,
                idx_lo_sb=idx_lo_sb, idx_hi_sb=idx_hi_sb,
                dstoff=dstoff.astype(ml_dtypes.bfloat16),
                dinv_slab=dinv_slab, dinv2_slab=dinv2_slab,
                sdeg_rows=sdeg_rows, batchoff=batchoff, g0=g0,
                iota_bf=iota_bf, iota_f32=iota_f32, xhat=xhat, cnt_g=cnt_g)


# ---------------------------------------------------------------- device

def build_layer(pre, last_layer: bool, reps: int = 1, bf16_table: bool = True):
    """One SPMD program: grouped bulk gathers + SpMM + dense matmul per tile.
    last_layer=False: hhat = relu(dinv^2 * (zW) + dinv*b)  [= dinv*h1 table]
    last_layer=True:  h2 = dinv * zW + b, pooled[goff] += sum h2
    """
    tiles = pre['tiles_per_core']
    C_lo, C_hi = pre['C_lo'], pre['C_hi']
    c_max = pre['c_max']
    sum_ca = pre['sum_ca']
    L_lo, L_hi = pre['L_lo'], pre['L_hi']
    lo_pos, hi_pos = pre['lo_pos'], pre['hi_pos']
    npad = pre['npad']
    assert tiles % GT == 0
    lo_groups = _make_groups(C_lo, GATHER_BUDGET)
    hi_groups = _make_groups(C_hi, GATHER_BUDGET)
    lo_start = {g[0]: gi for gi, g in enumerate(lo_groups)}
    hi_start = {g[0]: gi for gi, g in enumerate(hi_groups)}

    TDT = BF16 if bf16_table else F32
    nc = bacc.Bacc(get_trn_type() or "TRN2", target_bir_lowering=False,
                   debug=False)
    table_lo = nc.dram_tensor("table_lo", [SPLIT, D], TDT, kind="ExternalInput").ap()
    table_hi = nc.dram_tensor("table_hi", [npad - SPLIT, D], TDT,
                              kind="ExternalInput").ap()
    self_rows = nc.dram_tensor("self_rows", [tiles * P, D], TDT,
                               kind="ExternalInput").ap()
    idx_lo = nc.dram_tensor("idx_lo", [P, L_lo // 16], I16, kind="ExternalInput").ap()
    idx_hi = nc.dram_tensor("idx_hi", [P, L_hi // 16], I16, kind="ExternalInput").ap()
    dstoff = nc.dram_tensor("dstoff", [P, sum_ca], BF16, kind="ExternalInput").ap()
    iota_b = nc.dram_tensor("iota_b", [P, (c_max + 1) * D], BF16,
                            kind="ExternalInput").ap()
    iota_f = nc.dram_tensor("iota_f", [P, D], F32, kind="ExternalInput").ap()
    Wt = nc.dram_tensor("W", [D, D], F32, kind="ExternalInput").ap()
    bt = nc.dram_tensor("b", [1, D], F32, kind="ExternalInput").ap()
    dinv = nc.dram_tensor("dinv", [P, tiles], F32, kind="ExternalInput").ap()
    sdeg = nc.dram_tensor("sdeg", [1, tiles * P], F32, kind="ExternalInput").ap()
    if last_layer:
        batchoff = nc.dram_tensor("batchoff", [P, tiles], F32,
                                  kind="ExternalInput").ap()
        pooled = nc.dram_tensor("pooled", [P, D], F32, kind="ExternalOutput").ap()
    else:
        hhat = nc.dram_tensor("hhat", [tiles * P, D], TDT, kind="ExternalOutput").ap()

    max_lo = max(g[3] // P for g in lo_groups)
    max_hi = max(g[3] // P for g in hi_groups)

    with tile.TileContext(nc) as tc:
        with tc.tile_pool(name="const", bufs=1) as cp, \
             tc.tile_pool(name="glo", bufs=2) as glp, \
             tc.tile_pool(name="ghi", bufs=2) as ghp, \
             tc.tile_pool(name="gself", bufs=2) as gsp, \
             tc.tile_pool(name="sel", bufs=3) as sp, \
             tc.tile_pool(name="small", bufs=3) as mp, \
             tc.tile_pool(name="hout", bufs=2) as hp, \
             tc.tile_pool(name="ps1", bufs=2, space="PSUM") as pp1, \
             tc.tile_pool(name="ps2", bufs=2, space="PSUM") as pp2:
            ilo_t = cp.tile([P, L_lo // 16], I16)
            ihi_t = cp.tile([P, L_hi // 16], I16)
            dst_t = cp.tile([P, sum_ca], BF16)
            iob_t = cp.tile([P, (c_max + 1) * D], BF16)
            iof_t = cp.tile([P, D], F32)
            W_t = cp.tile([D, D], F32)
            b_t = cp.tile([1, D], F32)
            sdeg_t = cp.tile([1, tiles * P], F32)
            dinv_t = cp.tile([P, tiles], F32)
            nc.sync.dma_start(out=ilo_t[:], in_=idx_lo[:])
            nc.sync.dma_start(out=ihi_t[:], in_=idx_hi[:])
            nc.sync.dma_start(out=dst_t[:], in_=dstoff[:])
            nc.sync.dma_start(out=iob_t[:], in_=iota_b[:])
            nc.sync.dma_start(out=iof_t[:], in_=iota_f[:])
            nc.sync.dma_start(out=W_t[:], in_=Wt[:])
            nc.sync.dma_start(out=b_t[:], in_=bt[:])
            nc.sync.dma_start(out=sdeg_t[:], in_=sdeg[:])
            nc.sync.dma_start(out=dinv_t[:], in_=dinv[:])
            if last_layer:
                boff_t = cp.tile([P, tiles], F32)
                nc.sync.dma_start(out=boff_t[:], in_=batchoff[:])
                pooled_sb = cp.tile([P, D], F32)

            for rep in range(reps):
                if last_layer:
                    nc.vector.memset(pooled_sb[:], 0.0)
                cola = 0
                glo = ghi = gself = hh = None
                off_lo = off_hi = 0
                for t in range(tiles):
                    if t in lo_start:
                        ts, te, slot0, n_idx = lo_groups[lo_start[t]]
                        glo = glp.tile([P, max_lo * D], TDT, tag="glo")
                        off_lo = 0
                        if n_idx:
                            nc.gpsimd.dma_gather(
                                glo[:, :(n_idx // P) * D].rearrange(
                                    "p (c j) -> p c j", j=D),
                                table_lo[:],
                                ilo_t[:, slot0 // 16:(slot0 + n_idx) // 16],
                                n_idx, n_idx, D, single_packet=SINGLE_PACKET)
                    if t in hi_start:
                        ts, te, slot0, n_idx = hi_groups[hi_start[t]]
                        ghi = ghp.tile([P, max_hi * D], TDT, tag="ghi")
                        off_hi = 0
                        if n_idx:
                            nc.gpsimd.dma_gather(
                                ghi[:, :(n_idx // P) * D].rearrange(
                                    "p (c j) -> p c j", j=D),
                                table_hi[:],
                                ihi_t[:, slot0 // 16:(slot0 + n_idx) // 16],
                                n_idx, n_idx, D, single_packet=SINGLE_PACKET)
                    if t % GT == 0:
                        t0 = t
                        gself = gsp.tile([P, GT * D], TDT, tag="gs")
                        nc.sync.dma_start(
                            out=gself[:].rearrange("p (t d) -> p t d", d=D),
                            in_=self_rows[t0 * P:(t0 + GT) * P, :].rearrange(
                                "(t p) d -> p t d", p=P))
                        if not last_layer:
                            hh = hp.tile([P, GT * D], TDT, tag="hh")
                    Cl, Ch = int(C_lo[t]), int(C_hi[t])
                    C = Cl + Ch
                    S = sp.tile([P, (c_max + 1) * D], BF16, tag="s")
                    nc.vector.tensor_tensor(
                        out=S[:, :(C + 1) * D].rearrange("p (c j) -> p c j", j=D),
                        in0=dst_t[:, cola:cola + C + 1].to_broadcast([P, C + 1, D]),
                        in1=iob_t[:, :(C + 1) * D].rearrange("p (c j) -> p c j", j=D),
                        op=mybir.AluOpType.is_equal)
                    psumT = pp1.tile([P, D], F32, space="PSUM", tag="pT")
                    for c in range(C + 1):
                        if c < Cl:
                            lhsT = glo[:, (off_lo + c) * D:(off_lo + c + 1) * D]
                        elif c < C:
                            lhsT = ghi[:, (off_hi + c - Cl) * D:
                                       (off_hi + c - Cl + 1) * D]
                        else:
                            lhsT = gself[:, (t - t0) * D:(t - t0 + 1) * D]
                        nc.tensor.matmul(out=psumT[:], lhsT=lhsT,
                                         rhs=S[:, c * D:(c + 1) * D],
                                         start=(c == 0), stop=(c == C))
                    lhs_sb = mp.tile([P, D], F32, tag="lhs")
                    nc.scalar.activation(out=lhs_sb[:], in_=psumT[:],
                                         func=mybir.ActivationFunctionType.Copy)
                    psum2 = pp2.tile([P, D], F32, space="PSUM", tag="p2")
                    nc.tensor.matmul(out=psum2[:], lhsT=lhs_sb[:], rhs=W_t[:],
                                     start=True, stop=False)
                    nc.tensor.matmul(out=psum2[:],
                                     lhsT=sdeg_t[:, t * P:(t + 1) * P],
                                     rhs=b_t[:], start=False, stop=True)
                    if last_layer:
                        out_sb = mp.tile([P, D], F32, tag="out")
                        nc.scalar.activation(out=out_sb[:], in_=psum2[:],
                                             func=mybir.ActivationFunctionType.Copy,
                                             scale=dinv_t[:, t:t + 1])
                        Pt = sp.tile([P, D], F32, tag="pool_sel")
                        nc.vector.tensor_tensor(
                            out=Pt[:],
                            in0=boff_t[:, t:t + 1].to_broadcast([P, D]),
                            in1=iof_t[:],
                            op=mybir.AluOpType.is_equal)
                        pool_ps = pp2.tile([P, D], F32, space="PSUM", tag="pool_ps")
                        nc.tensor.matmul(out=pool_ps[:], lhsT=Pt[:], rhs=out_sb[:],
                                         start=True, stop=True)
                        nc.vector.tensor_add(out=pooled_sb[:], in0=pooled_sb[:],
                                             in1=pool_ps[:])
                    else:
                        # hh = relu(dinv^2 * zW + dinv*b) = dinv * h1
                        nc.scalar.activation(out=hh[:, (t - t0) * D:(t - t0 + 1) * D],
                                             in_=psum2[:],
                                             func=mybir.ActivationFunctionType.Relu,
                                             scale=dinv_t[:, t:t + 1])
                    cola += C + 1
                    off_lo += Cl
                    off_hi += Ch
                    if not last_layer and t % GT == GT - 1:
                        nc.sync.dma_start(
                            out=hhat[t0 * P:(t0 + GT) * P, :].rearrange(
                                "(t p) d -> p t d", p=P),
                            in_=hh[:].rearrange("p (t d) -> p t d", d=D))
                if last_layer:
                    nc.sync.dma_start(out=pooled[:], in_=pooled_sb[:])
    nc.compile()
    return nc


def _in_maps(pre, table_np, W, b, last_layer):
    maps = []
    rpc = pre['rows_per_core']
    t_lo = np.ascontiguousarray(table_np[:SPLIT])
    t_hi = np.ascontiguousarray(table_np[SPLIT:])
    for k in range(N_CORES):
        m = dict(table_lo=t_lo, table_hi=t_hi,
                 self_rows=np.ascontiguousarray(table_np[k * rpc:(k + 1) * rpc]),
                 idx_lo=pre['idx_lo_sb'][k],
                 idx_hi=pre['idx_hi_sb'][k],
                 dstoff=pre['dstoff'][k],
                 iota_b=pre['iota_bf'],
                 iota_f=pre['iota_f32'],
                 W=np.ascontiguousarray(W, dtype=np.float32),
                 b=np.ascontiguousarray(b, dtype=np.float32).reshape(1, D),
                 dinv=pre['dinv_slab'][k] if last_layer else pre['dinv2_slab'][k],
                 sdeg=pre['sdeg_rows'][k])
        if last_layer:
            m['batchoff'] = pre['batchoff'][k]
        maps.append(m)
    return maps


def kernel(x, edge_index, batch, W1, b1, W2, b2):
    x = np.asarray(x); edge_index = np.asarray(edge_index)
    batch = np.asarray(batch)
    W1 = np.asarray(W1); b1 = np.asarray(b1)
    W2 = np.asarray(W2); b2 = np.asarray(b2)

    pre = preprocess(x, edge_index, batch)
    core_ids = list(range(N_CORES))

    tdt = ml_dtypes.bfloat16 if USE_BF16 else np.float32
    table1 = pre['xhat'].astype(tdt)
    nc1 = build_layer(pre, last_layer=False, bf16_table=USE_BF16)
    res1 = run_bass_kernel_spmd(nc1, _in_maps(pre, table1, W1, b1, False),
                                core_ids).results

    h1hat = np.zeros((pre['npad'], D), dtype=tdt)
    rpc = pre['rows_per_core']
    for k in range(N_CORES):
        h1hat[k * rpc:(k + 1) * rpc] = res1[k]['hhat']

    nc2 = build_layer(pre, last_layer=True, bf16_table=USE_BF16)
    res2 = run_bass_kernel_spmd(nc2, _in_maps(pre, h1hat, W2, b2, True),
                                core_ids).results

    pooled = np.zeros((G, D), dtype=np.float32)
    for k in range(N_CORES):
        part = res2[k]['pooled']
        g0 = int(pre['g0'][k])
        span = min(P, G - g0)
        pooled[g0:g0 + span] += part[:span]
    return pooled / np.maximum(pre['cnt_g'], 1.0)[:, None]


# revision 45
# speedup vs baseline: 4.8182x; 2.7166x over previous
"""Trainium2 Bass kernel for a 2-layer GCN encoder (GCNConv x2 + global mean pool).

Math: with A' = A + I and deg = indegree(A') (symmetric-norm GCN),
    gcn(h, W, b) = D^-1/2 A' D^-1/2 (h) W + b
factorized as  out = dinv * (A' @ (dinv * h)) @ W + b   (dinv = deg^-1/2)
so the SpMM is pure 0/1 structure; per-edge norms become per-node row scales.

Sharding: dst-node ranges across 8 cores (6272 padded rows each). Each core:
  - bulk-gathers source rows of the (replicated) scaled feature table from HBM
    with InstDMAGatherAnt (one SWDGE op per ~7-tile group instead of one
    indirect op per 128 rows: the ~1us/op descriptor-generation fixed cost
    dominated the old kernel). Indices are int16, so the table is split at row
    32768 into lo/hi halves with one gather each per group,
  - streams each tile's self-loop block with one regular HWDGE DMA per group,
  - builds the 0/1 scatter matrix on VectorE in bf16 (compare dst-offset vs
    iota; the self-loop chunk's column is an identity pattern),
  - scatter-adds via TensorE one-hot matmuls accumulating in PSUM (psum_T is
    [din, dstoff], already transposed for the dense W matmul),
  - dense W matmul + rank-1 bias matmul; ScalarE applies relu and the row
    scale (layer 1 uses scale=dinv^2, folding both dinv factors around the
    relu: dinv*relu(dinv*zW + b) == relu(dinv^2*zW*... ) since dinv > 0).
Two SPMD launches (layer 1 -> host allgather of the 1.6MB/core slabs ->
layer 2 + graph pooling via one-hot matmul over sorted batch ids).
"""
import math
import numpy as np
import ml_dtypes

from concourse import bass, mybir, tile, bacc
from concourse.bass_utils import run_bass_kernel_spmd
from concourse._compat import get_trn_type

N_CORES = 8
P = 128          # partitions / tile rows
D = 128          # feature dim
G = 512          # number of graphs (fixed by the problem)
SPLIT = 32768    # dma_gather int16 index limit: table split row
GT = 7           # dst tiles per self-rows DMA group (49 = 7 x 7)
GATHER_BUDGET = 896   # 56+1 descs/lane: under the 64-desc packet ceiling
SINGLE_PACKET = True  # one packet per lane per op drains far faster
N_SWDGE_Q = 4         # round-robin gathers over SWDGE queues: each SDMA
                      # engine interleaves queue contexts, ~4x gather rate
F32 = mybir.dt.float32
BF16 = mybir.dt.bfloat16
I16 = mybir.dt.int16

USE_BF16 = True     # bf16 gather table (half the random-gather bytes)


def _windows(total_chunks, budget_slots):
    """Slice a chunk stream into fixed-size gather windows (last one ragged).
    Windows are tile-agnostic: a tile's chunks may span two windows."""
    W = budget_slots // P
    return [(w * W, min(W, total_chunks - w * W))
            for w in range((total_chunks + W - 1) // W)]


# ---------------------------------------------------------------- host prep

def preprocess(x, edge_index, batch):
    N = x.shape[0]
    rows_per_core = int(math.ceil(N / (N_CORES * P))) * P
    npad = rows_per_core * N_CORES
    tiles = rows_per_core // P
    n_tiles_g = N_CORES * tiles

    src = edge_index[0].astype(np.int64)
    dst = edge_index[1].astype(np.int64)
    # Self-loops are handled as a per-tile sequential-DMA chunk (no gather
    # slots); only real edges go through the sorted streams.
    deg = (np.bincount(dst, minlength=N) + 1).astype(np.float32)
    dinv = 1.0 / np.sqrt(np.maximum(deg, 1.0))

    xhat = np.zeros((npad, D), dtype=np.float32)
    xhat[:N] = x.astype(np.float32) * dinv[:, None]

    order = np.argsort(dst, kind="stable")
    src_s = src[order].astype(np.int64)
    dst_s = dst[order]
    bounds = np.searchsorted(dst_s, np.arange(0, npad + 1, P))

    # per-global-tile lo/hi edge partition (src < SPLIT goes to the lo gather)
    lo_srcs, lo_offs, hi_srcs, hi_offs = [], [], [], []
    n_lo = np.zeros(n_tiles_g, dtype=np.int64)
    n_hi = np.zeros(n_tiles_g, dtype=np.int64)
    for gt in range(n_tiles_g):
        s, e = int(bounds[gt]), int(bounds[gt + 1])
        seg_src = src_s[s:e]
        seg_off = (dst_s[s:e] - gt * P).astype(np.float32)
        m = seg_src < SPLIT
        lo_srcs.append(seg_src[m]); lo_offs.append(seg_off[m])
        hi_srcs.append(seg_src[~m] - SPLIT); hi_offs.append(seg_off[~m])
        n_lo[gt] = int(m.sum()); n_hi[gt] = int((~m).sum())

    C_lo = np.ceil(n_lo.reshape(N_CORES, tiles).max(axis=0) / P).astype(np.int64)
    C_hi = np.ceil(n_hi.reshape(N_CORES, tiles).max(axis=0) / P).astype(np.int64)
    c_tot = C_lo + C_hi                      # chunks per tile w/o self chunk
    c_max = int(c_tot.max())
    sum_ca = int(c_tot.sum()) + tiles        # dstoff cols incl self chunk

    L_lo = int(C_lo.sum()) * P               # lo gather slots per core
    L_hi = int(C_hi.sum()) * P
    lo_pos = np.concatenate([[0], np.cumsum(C_lo * P)])
    hi_pos = np.concatenate([[0], np.cumsum(C_hi * P)])

    idx_lo = np.zeros((N_CORES, L_lo), dtype=np.int32)
    idx_hi = np.zeros((N_CORES, L_hi), dtype=np.int32)
    dstoff = np.full((N_CORES, P, sum_ca), float(P), dtype=np.float32)
    for k in range(N_CORES):
        cola = 0
        for t in range(tiles):
            gt = k * tiles + t
            Cl, Ch = int(C_lo[t]), int(C_hi[t])
            nl, nh = int(n_lo[gt]), int(n_hi[gt])
            if nl:
                idx_lo[k, lo_pos[t]:lo_pos[t] + nl] = lo_srcs[gt]
                bo = np.full(Cl * P, float(P), dtype=np.float32)
                bo[:nl] = lo_offs[gt]
                dstoff[k, :, cola:cola + Cl] = bo.reshape(Cl, P).T
            cola += Cl
            if nh:
                idx_hi[k, hi_pos[t]:hi_pos[t] + nh] = hi_srcs[gt]
                bo = np.full(Ch * P, float(P), dtype=np.float32)
                bo[:nh] = hi_offs[gt]
                dstoff[k, :, cola:cola + Ch] = bo.reshape(Ch, P).T
            cola += Ch
            # identity column: self-loop chunk scatters partition e -> column e
            # (pad rows contribute zero because xhat is zero there)
            dstoff[k, :, cola] = np.arange(P, dtype=np.float32)
            cola += 1
    assert cola == sum_ca

    def idx_sbuf(a):
        # dma_gather idx layout: element i at partition i%16, col i//16,
        # replicated across all eight 16-partition stripes.
        sb = a.reshape(-1, 16).T
        return np.ascontiguousarray(np.tile(sb, (8, 1)).astype(np.int16))

    idx_lo_sb = np.stack([idx_sbuf(idx_lo[k]) for k in range(N_CORES)])
    idx_hi_sb = np.stack([idx_sbuf(idx_hi[k]) for k in range(N_CORES)])

    dinv_pad = np.zeros(npad, dtype=np.float32)
    dinv_pad[:N] = dinv
    dinv_slab = dinv_pad.reshape(N_CORES, tiles, P).transpose(0, 2, 1).copy()
    dinv2_slab = (dinv_slab * dinv_slab).copy()
    # sqrt(deg) per row (0 on pad rows): the bias is injected into PSUM as
    # sdeg[p]*b[j]; the later per-row scale turns it into the exact bias.
    sdeg_pad = np.zeros(npad, dtype=np.float32)
    sdeg_pad[:N] = np.sqrt(np.maximum(deg, 1.0))
    sdeg_rows = sdeg_pad.reshape(N_CORES, 1, tiles * P).copy()

    batch_pad = np.full(npad, -1, dtype=np.int64)
    batch_pad[:N] = batch.astype(np.int64)
    g0 = np.zeros(N_CORES, dtype=np.int64)
    batchoff = np.full((N_CORES, P, tiles), float(P), dtype=np.float32)
    for k in range(N_CORES):
        b = batch_pad[k * rows_per_core:(k + 1) * rows_per_core]
        real = b >= 0
        assert real.any()
        g0[k] = b[real].min()
        span = int(b[real].max() - g0[k]) + 1
        assert span <= P - 1, f"graph span {span} exceeds pooling tile"
        off = np.full(rows_per_core, float(P), dtype=np.float32)
        off[real] = (b[real] - g0[k]).astype(np.float32)
        batchoff[k] = off.reshape(tiles, P).T

    # j-major iota const: iota_jm[p, j*(c_max+1)+c] = j. The S build runs
    # j-outer/c-inner so every operand's innermost AP dim is unit-stride,
    # which is what unlocks the DVE 2x 16-bit mode.
    iota_jm = np.tile(np.repeat(np.arange(P, dtype=np.float32), c_max + 1),
                      (P, 1)).astype(ml_dtypes.bfloat16)
    # same j-major layout with a tile axis, for the batched pool-selector
    iota_pool = np.tile(np.repeat(np.arange(P, dtype=np.float32), tiles),
                        (P, 1)).astype(ml_dtypes.bfloat16)
    cnt_g = np.bincount(batch.astype(np.int64), minlength=G).astype(np.float32)

    return dict(N=N, npad=npad, rows_per_core=rows_per_core,
                tiles_per_core=tiles, C_lo=C_lo, C_hi=C_hi, c_max=c_max,
                sum_ca=sum_ca, L_lo=L_lo, L_hi=L_hi,
                lo_pos=lo_pos, hi_pos=hi_pos,
                idx_lo_sb=idx_lo_sb, idx_hi_sb=idx_hi_sb,
                dstoff=dstoff.astype(ml_dtypes.bfloat16),
                dinv_slab=dinv_slab, dinv2_slab=dinv2_slab,
                sdeg_rows=sdeg_rows, g0=g0,
                batchoff=batchoff.astype(ml_dtypes.bfloat16),
                iota_jm=iota_jm, iota_pool=iota_pool, xhat=xhat, cnt_g=cnt_g)


# ---------------------------------------------------------------- device

def build_layer(pre, last_layer: bool, reps: int = 1, bf16_table: bool = True):
    """One SPMD program: grouped bulk gathers + SpMM + dense matmul per tile.
    last_layer=False: hhat = relu(dinv^2 * (zW) + dinv*b)  [= dinv*h1 table]
    last_layer=True:  h2 = dinv * zW + b, pooled[goff] += sum h2
    """
    tiles = pre['tiles_per_core']
    C_lo, C_hi = pre['C_lo'], pre['C_hi']
    c_max = pre['c_max']
    sum_ca = pre['sum_ca']
    L_lo, L_hi = pre['L_lo'], pre['L_hi']
    lo_pos, hi_pos = pre['lo_pos'], pre['hi_pos']
    npad = pre['npad']
    assert tiles % GT == 0
    sumC_lo, sumC_hi = int(C_lo.sum()), int(C_hi.sum())
    lo_wins = _windows(sumC_lo, GATHER_BUDGET)
    hi_wins = _windows(sumC_hi, GATHER_BUDGET)
    WCH = GATHER_BUDGET // P      # chunks per full window
    lo_cstart = np.concatenate([[0], np.cumsum(C_lo)])  # global lo chunk idx
    hi_cstart = np.concatenate([[0], np.cumsum(C_hi)])

    TDT = BF16 if bf16_table else F32
    nc = bacc.Bacc(get_trn_type() or "TRN2", target_bir_lowering=False,
                   debug=False, num_swdge_queues=N_SWDGE_Q)
    table_lo = nc.dram_tensor("table_lo", [SPLIT, D], TDT, kind="ExternalInput").ap()
    table_hi = nc.dram_tensor("table_hi", [npad - SPLIT, D], TDT,
                              kind="ExternalInput").ap()
    self_rows = nc.dram_tensor("self_rows", [tiles * P, D], TDT,
                               kind="ExternalInput").ap()
    idx_lo = nc.dram_tensor("idx_lo", [P, L_lo // 16], I16, kind="ExternalInput").ap()
    idx_hi = nc.dram_tensor("idx_hi", [P, L_hi // 16], I16, kind="ExternalInput").ap()
    dstoff = nc.dram_tensor("dstoff", [P, sum_ca], BF16, kind="ExternalInput").ap()
    iota_b = nc.dram_tensor("iota_b", [P, (c_max + 1) * D], BF16,
                            kind="ExternalInput").ap()
    if last_layer:
        iota_p = nc.dram_tensor("iota_p", [P, tiles * D], BF16,
                                kind="ExternalInput").ap()
    Wt = nc.dram_tensor("W", [D, D], F32, kind="ExternalInput").ap()
    bt = nc.dram_tensor("b", [1, D], F32, kind="ExternalInput").ap()
    dinv = nc.dram_tensor("dinv", [P, tiles], F32, kind="ExternalInput").ap()
    sdeg = nc.dram_tensor("sdeg", [1, tiles * P], F32, kind="ExternalInput").ap()
    if last_layer:
        batchoff = nc.dram_tensor("batchoff", [P, tiles], BF16,
                                  kind="ExternalInput").ap()
        pooled = nc.dram_tensor("pooled", [P, D], F32, kind="ExternalOutput").ap()
    else:
        hhat = nc.dram_tensor("hhat", [tiles * P, D], TDT, kind="ExternalOutput").ap()



    with tile.TileContext(nc) as tc:
        with tc.tile_pool(name="const", bufs=1) as cp, \
             tc.tile_pool(name="glo", bufs=6) as glp, \
             tc.tile_pool(name="ghi", bufs=6) as ghp, \
             tc.tile_pool(name="gself", bufs=2) as gsp, \
             tc.tile_pool(name="sel", bufs=3) as sp, \
             tc.tile_pool(name="small", bufs=3) as mp, \
             tc.tile_pool(name="hout", bufs=2) as hp, \
             tc.tile_pool(name="ps1", bufs=2, space="PSUM") as pp1, \
             tc.tile_pool(name="ps2", bufs=2, space="PSUM") as pp2, \
             tc.tile_pool(name="ps3", bufs=2, space="PSUM") as pp3:
            ilo_t = cp.tile([P, L_lo // 16], I16)
            ihi_t = cp.tile([P, L_hi // 16], I16)
            dst_t = cp.tile([P, sum_ca], BF16)
            iob_t = cp.tile([P, (c_max + 1) * D], BF16)
            W_t = cp.tile([D, D], F32)
            b_t = cp.tile([1, D], F32)
            sdeg_t = cp.tile([1, tiles * P], F32)
            dinv_t = cp.tile([P, tiles], F32)
            nc.sync.dma_start(out=ilo_t[:], in_=idx_lo[:])
            nc.sync.dma_start(out=ihi_t[:], in_=idx_hi[:])
            nc.sync.dma_start(out=dst_t[:], in_=dstoff[:])
            nc.sync.dma_start(out=iob_t[:], in_=iota_b[:])
            nc.sync.dma_start(out=W_t[:], in_=Wt[:])
            nc.sync.dma_start(out=b_t[:], in_=bt[:])
            nc.sync.dma_start(out=sdeg_t[:], in_=sdeg[:])
            nc.sync.dma_start(out=dinv_t[:], in_=dinv[:])
            if last_layer:
                boff_t = cp.tile([P, tiles], BF16)
                iop_t = cp.tile([P, tiles * D], BF16)
                nc.sync.dma_start(out=boff_t[:], in_=batchoff[:])
                nc.sync.dma_start(out=iop_t[:], in_=iota_p[:])
                # all 49 pool one-hot selectors in one 2x-mode DVE op
                ptall = cp.tile([P, tiles * D], BF16)
                ptall_cm = ptall[:].rearrange("p (j t) -> p t j", t=tiles)
                nc.vector.tensor_tensor(
                    out=ptall[:].rearrange("p (j t) -> p j t", t=tiles),
                    in0=boff_t[:].rearrange("p (t o) -> p o t",
                                            o=1).to_broadcast([P, D, tiles]),
                    in1=iop_t[:].rearrange("p (j t) -> p j t", t=tiles),
                    op=mybir.AluOpType.is_equal)

            for rep in range(reps):
                if last_layer:
                    pool_acc = pp3.tile([P, D], F32, space="PSUM", tag="pacc")
                cola = 0
                gself = hh = None
                gq = 0
                lo_bufs, hi_bufs = {}, {}
                lo_next = hi_next = 0
                for t in range(tiles):
                    # issue gather windows covering this tile's chunks
                    while (lo_next < len(lo_wins)
                           and lo_wins[lo_next][0] < lo_cstart[t + 1]):
                        c0, nch = lo_wins[lo_next]
                        buf = glp.tile([P, WCH * D], TDT, tag="glo")
                        nc.gpsimd.dma_gather(
                            buf[:, :nch * D].rearrange("p (c j) -> p c j", j=D),
                            table_lo[:],
                            ilo_t[:, c0 * P // 16:(c0 + nch) * P // 16],
                            nch * P, nch * P, D, single_packet=SINGLE_PACKET,
                            queue_num=gq % N_SWDGE_Q)
                        gq += 1
                        lo_bufs[lo_next] = buf
                        lo_next += 1
                    while (hi_next < len(hi_wins)
                           and hi_wins[hi_next][0] < hi_cstart[t + 1]):
                        c0, nch = hi_wins[hi_next]
                        buf = ghp.tile([P, WCH * D], TDT, tag="ghi")
                        nc.gpsimd.dma_gather(
                            buf[:, :nch * D].rearrange("p (c j) -> p c j", j=D),
                            table_hi[:],
                            ihi_t[:, c0 * P // 16:(c0 + nch) * P // 16],
                            nch * P, nch * P, D, single_packet=SINGLE_PACKET,
                            queue_num=gq % N_SWDGE_Q)
                        gq += 1
                        hi_bufs[hi_next] = buf
                        hi_next += 1
                    if t % GT == 0:
                        t0 = t
                        gself = gsp.tile([P, GT * D], TDT, tag="gs")
                        nc.sync.dma_start(
                            out=gself[:].rearrange("p (t d) -> p t d", d=D),
                            in_=self_rows[t0 * P:(t0 + GT) * P, :].rearrange(
                                "(t p) d -> p t d", p=P))
                        if not last_layer:
                            hh = hp.tile([P, GT * D], TDT, tag="hh")
                    Cl, Ch = int(C_lo[t]), int(C_hi[t])
                    C = Cl + Ch
                    # S built j-major (S[p, j, c]) so every operand's innermost
                    # AP dim is unit-stride -> DVE 2x 16-bit mode. The matmul
                    # then reads chunk c as a stride-(C+1) rhs slice.
                    S = sp.tile([P, (c_max + 1) * D], BF16, tag="s")
                    nc.vector.tensor_tensor(
                        out=S[:, :(C + 1) * D].rearrange("p (j c) -> p j c",
                                                         c=C + 1),
                        in0=dst_t[:, cola:cola + C + 1].rearrange(
                            "p (c o) -> p o c", o=1).to_broadcast([P, D, C + 1]),
                        in1=iob_t[:].rearrange("p (j c) -> p j c",
                                               c=c_max + 1)[:, :, :C + 1],
                        op=mybir.AluOpType.is_equal)
                    S_cm = S[:, :(C + 1) * D].rearrange("p (j c) -> p c j",
                                                        c=C + 1)
                    psumT = pp1.tile([P, D], F32, space="PSUM", tag="pT")
                    for c in range(C + 1):
                        if c < Cl:
                            q = int(lo_cstart[t]) + c
                            lhsT = lo_bufs[q // WCH][:, (q % WCH) * D:
                                                     (q % WCH + 1) * D]
                        elif c < C:
                            q = int(hi_cstart[t]) + (c - Cl)
                            lhsT = hi_bufs[q // WCH][:, (q % WCH) * D:
                                                     (q % WCH + 1) * D]
                        else:
                            lhsT = gself[:, (t - t0) * D:(t - t0 + 1) * D]
                        nc.tensor.matmul(out=psumT[:], lhsT=lhsT,
                                         rhs=S_cm[:, c],
                                         start=(c == 0), stop=(c == C))
                    lhs_sb = mp.tile([P, D], F32, tag="lhs")
                    nc.scalar.activation(out=lhs_sb[:], in_=psumT[:],
                                         func=mybir.ActivationFunctionType.Copy)
                    psum2 = pp2.tile([P, D], F32, space="PSUM", tag="p2")
                    nc.tensor.matmul(out=psum2[:], lhsT=lhs_sb[:], rhs=W_t[:],
                                     start=True, stop=False)
                    nc.tensor.matmul(out=psum2[:],
                                     lhsT=sdeg_t[:, t * P:(t + 1) * P],
                                     rhs=b_t[:], start=False, stop=True)
                    if last_layer:
                        out_sb = mp.tile([P, D], BF16, tag="out")
                        nc.scalar.activation(out=out_sb[:], in_=psum2[:],
                                             func=mybir.ActivationFunctionType.Copy,
                                             scale=dinv_t[:, t:t + 1])
                        nc.tensor.matmul(out=pool_acc[:], lhsT=ptall_cm[:, t],
                                         rhs=out_sb[:],
                                         start=(t == 0), stop=(t == tiles - 1))
                    else:
                        # hh = relu(dinv^2 * zW + dinv*b) = dinv * h1
                        nc.scalar.activation(out=hh[:, (t - t0) * D:(t - t0 + 1) * D],
                                             in_=psum2[:],
                                             func=mybir.ActivationFunctionType.Relu,
                                             scale=dinv_t[:, t:t + 1])
                    cola += C + 1
                    if not last_layer and t % GT == GT - 1:
                        nc.sync.dma_start(
                            out=hhat[t0 * P:(t0 + GT) * P, :].rearrange(
                                "(t p) d -> p t d", p=P),
                            in_=hh[:].rearrange("p (t d) -> p t d", d=D))
                if last_layer:
                    pooled_sb = mp.tile([P, D], F32, tag="pool_out")
                    nc.scalar.activation(out=pooled_sb[:], in_=pool_acc[:],
                                         func=mybir.ActivationFunctionType.Copy)
                    nc.sync.dma_start(out=pooled[:], in_=pooled_sb[:])
    nc.compile()
    return nc


def _in_maps(pre, table_np, W, b, last_layer):
    maps = []
    rpc = pre['rows_per_core']
    t_lo = np.ascontiguousarray(table_np[:SPLIT])
    t_hi = np.ascontiguousarray(table_np[SPLIT:])
    for k in range(N_CORES):
        m = dict(table_lo=t_lo, table_hi=t_hi,
                 self_rows=np.ascontiguousarray(table_np[k * rpc:(k + 1) * rpc]),
                 idx_lo=pre['idx_lo_sb'][k],
                 idx_hi=pre['idx_hi_sb'][k],
                 dstoff=pre['dstoff'][k],
                 iota_b=pre['iota_jm'],
                 **({'iota_p': pre['iota_pool']} if last_layer else {}),
                 W=np.ascontiguousarray(W, dtype=np.float32),
                 b=np.ascontiguousarray(b, dtype=np.float32).reshape(1, D),
                 dinv=pre['dinv_slab'][k] if last_layer else pre['dinv2_slab'][k],
                 sdeg=pre['sdeg_rows'][k])
        if last_layer:
            m['batchoff'] = pre['batchoff'][k]
        maps.append(m)
    return maps


def kernel(x, edge_index, batch, W1, b1, W2, b2):
    x = np.asarray(x); edge_index = np.asarray(edge_index)
    batch = np.asarray(batch)
    W1 = np.asarray(W1); b1 = np.asarray(b1)
    W2 = np.asarray(W2); b2 = np.asarray(b2)

    pre = preprocess(x, edge_index, batch)
    core_ids = list(range(N_CORES))

    tdt = ml_dtypes.bfloat16 if USE_BF16 else np.float32
    table1 = pre['xhat'].astype(tdt)
    nc1 = build_layer(pre, last_layer=False, bf16_table=USE_BF16)
    res1 = run_bass_kernel_spmd(nc1, _in_maps(pre, table1, W1, b1, False),
                                core_ids).results

    h1hat = np.zeros((pre['npad'], D), dtype=tdt)
    rpc = pre['rows_per_core']
    for k in range(N_CORES):
        h1hat[k * rpc:(k + 1) * rpc] = res1[k]['hhat']

    nc2 = build_layer(pre, last_layer=True, bf16_table=USE_BF16)
    res2 = run_bass_kernel_spmd(nc2, _in_maps(pre, h1hat, W2, b2, True),
                                core_ids).results

    pooled = np.zeros((G, D), dtype=np.float32)
    for k in range(N_CORES):
        part = res2[k]['pooled']
        g0 = int(pre['g0'][k])
        span = min(P, G - g0)
        pooled[g0:g0 + span] += part[:span]
    return pooled / np.maximum(pre['cnt_g'], 1.0)[:, None]


# revision 59
# speedup vs baseline: 6.0612x; 1.2580x over previous
"""Trainium2 Bass kernel for a 2-layer GCN encoder (GCNConv x2 + global mean pool).

Math: with A' = A + I and deg = indegree(A') (symmetric-norm GCN),
    gcn(h, W, b) = D^-1/2 A' D^-1/2 (h) W + b
factorized as  out = dinv * (A' @ (dinv * h)) @ W + b   (dinv = deg^-1/2)
so the SpMM is pure 0/1 structure; per-edge norms become per-node row scales.

Sharding: dst-node ranges across 8 cores (6272 padded rows each). Each core:
  - bulk-gathers source rows of the (replicated) scaled feature table from HBM
    with InstDMAGatherAnt (one SWDGE op per ~7-tile group instead of one
    indirect op per 128 rows: the ~1us/op descriptor-generation fixed cost
    dominated the old kernel). Indices are int16, so the table is split at row
    32768 into lo/hi halves with one gather each per group,
  - streams each tile's self-loop block with one regular HWDGE DMA per group,
  - builds the 0/1 scatter matrix on VectorE in bf16 (compare dst-offset vs
    iota; the self-loop chunk's column is an identity pattern),
  - scatter-adds via TensorE one-hot matmuls accumulating in PSUM (psum_T is
    [din, dstoff], already transposed for the dense W matmul),
  - dense W matmul + rank-1 bias matmul; ScalarE applies relu and the row
    scale (layer 1 uses scale=dinv^2, folding both dinv factors around the
    relu: dinv*relu(dinv*zW + b) == relu(dinv^2*zW*... ) since dinv > 0).
Two SPMD launches (layer 1 -> host allgather of the 1.6MB/core slabs ->
layer 2 + graph pooling via one-hot matmul over sorted batch ids).
"""
import math
import numpy as np
import ml_dtypes

from concourse import bass, mybir, tile, bacc
from concourse.bass_utils import run_bass_kernel_spmd
from concourse._compat import get_trn_type

N_CORES = 8
P = 128          # partitions / tile rows
D = 128          # feature dim
G = 512          # number of graphs (fixed by the problem)
SPLIT = 32768    # dma_gather int16 index limit: table split row
GT = 7           # dst tiles per self-rows DMA group (49 = 7 x 7)
GATHER_BUDGET = 896   # 56+1 descs/lane: under the 64-desc packet ceiling
SINGLE_PACKET = True  # one packet per lane per op drains far faster
N_SWDGE_Q = 4         # round-robin gathers over SWDGE queues: each SDMA
                      # engine interleaves queue contexts, ~4x gather rate
F32 = mybir.dt.float32
BF16 = mybir.dt.bfloat16
I16 = mybir.dt.int16

USE_BF16 = True     # bf16 gather table (half the random-gather bytes)


def _windows(total_chunks, budget_slots):
    """Slice a chunk stream into fixed-size gather windows (last one ragged).
    Windows are tile-agnostic: a tile's chunks may span two windows."""
    W = budget_slots // P
    return [(w * W, min(W, total_chunks - w * W))
            for w in range((total_chunks + W - 1) // W)]


# ---------------------------------------------------------------- host prep

def preprocess(x, edge_index, batch):
    N = x.shape[0]
    rows_per_core = int(math.ceil(N / (N_CORES * P))) * P
    npad = rows_per_core * N_CORES
    tiles = rows_per_core // P
    n_tiles_g = N_CORES * tiles

    src = edge_index[0].astype(np.int64)
    dst = edge_index[1].astype(np.int64)
    # Self-loops are handled as a per-tile sequential-DMA chunk (no gather
    # slots); only real edges go through the sorted streams.
    deg = (np.bincount(dst, minlength=N) + 1).astype(np.float32)
    dinv = 1.0 / np.sqrt(np.maximum(deg, 1.0))

    xhat = np.zeros((npad, D), dtype=np.float32)
    xhat[:N] = x.astype(np.float32) * dinv[:, None]

    order = np.argsort(dst, kind="stable")
    src_s = src[order].astype(np.int64)
    dst_s = dst[order]
    bounds = np.searchsorted(dst_s, np.arange(0, npad + 1, P))

    # per-global-tile lo/hi edge partition (src < SPLIT goes to the lo gather)
    lo_srcs, lo_offs, hi_srcs, hi_offs = [], [], [], []
    n_lo = np.zeros(n_tiles_g, dtype=np.int64)
    n_hi = np.zeros(n_tiles_g, dtype=np.int64)
    for gt in range(n_tiles_g):
        s, e = int(bounds[gt]), int(bounds[gt + 1])
        seg_src = src_s[s:e]
        seg_off = (dst_s[s:e] - gt * P).astype(np.float32)
        m = seg_src < SPLIT
        lo_srcs.append(seg_src[m]); lo_offs.append(seg_off[m])
        hi_srcs.append(seg_src[~m] - SPLIT); hi_offs.append(seg_off[~m])
        n_lo[gt] = int(m.sum()); n_hi[gt] = int((~m).sum())

    C_lo = np.ceil(n_lo.reshape(N_CORES, tiles).max(axis=0) / P).astype(np.int64)
    C_hi = np.ceil(n_hi.reshape(N_CORES, tiles).max(axis=0) / P).astype(np.int64)
    c_tot = C_lo + C_hi                      # chunks per tile w/o self chunk
    c_max = int(c_tot.max())
    sum_ca = int(c_tot.sum()) + tiles        # dstoff cols incl self chunk

    L_lo = int(C_lo.sum()) * P               # lo gather slots per core
    L_hi = int(C_hi.sum()) * P
    lo_pos = np.concatenate([[0], np.cumsum(C_lo * P)])
    hi_pos = np.concatenate([[0], np.cumsum(C_hi * P)])

    idx_lo = np.zeros((N_CORES, L_lo), dtype=np.int32)
    idx_hi = np.zeros((N_CORES, L_hi), dtype=np.int32)
    dstoff = np.full((N_CORES, P, sum_ca), float(P), dtype=np.float32)
    for k in range(N_CORES):
        cola = 0
        for t in range(tiles):
            gt = k * tiles + t
            Cl, Ch = int(C_lo[t]), int(C_hi[t])
            nl, nh = int(n_lo[gt]), int(n_hi[gt])
            if nl:
                idx_lo[k, lo_pos[t]:lo_pos[t] + nl] = lo_srcs[gt]
                bo = np.full(Cl * P, float(P), dtype=np.float32)
                bo[:nl] = lo_offs[gt]
                dstoff[k, :, cola:cola + Cl] = bo.reshape(Cl, P).T
            cola += Cl
            if nh:
                idx_hi[k, hi_pos[t]:hi_pos[t] + nh] = hi_srcs[gt]
                bo = np.full(Ch * P, float(P), dtype=np.float32)
                bo[:nh] = hi_offs[gt]
                dstoff[k, :, cola:cola + Ch] = bo.reshape(Ch, P).T
            cola += Ch
            # identity column: self-loop chunk scatters partition e -> column e
            # (pad rows contribute zero because xhat is zero there)
            dstoff[k, :, cola] = np.arange(P, dtype=np.float32)
            cola += 1
    assert cola == sum_ca

    def idx_sbuf(a):
        # dma_gather idx layout: element i at partition i%16, col i//16,
        # replicated across all eight 16-partition stripes.
        sb = a.reshape(-1, 16).T
        return np.ascontiguousarray(np.tile(sb, (8, 1)).astype(np.int16))

    idx_lo_sb = np.stack([idx_sbuf(idx_lo[k]) for k in range(N_CORES)])
    idx_hi_sb = np.stack([idx_sbuf(idx_hi[k]) for k in range(N_CORES)])

    dinv_pad = np.zeros(npad, dtype=np.float32)
    dinv_pad[:N] = dinv
    dinv_slab = dinv_pad.reshape(N_CORES, tiles, P).transpose(0, 2, 1).copy()
    dinv2_slab = (dinv_slab * dinv_slab).copy()
    # sqrt(deg) per row (0 on pad rows): the bias is injected into PSUM as
    # sdeg[p]*b[j]; the later per-row scale turns it into the exact bias.
    sdeg_pad = np.zeros(npad, dtype=np.float32)
    sdeg_pad[:N] = np.sqrt(np.maximum(deg, 1.0))
    sdeg_rows = sdeg_pad.reshape(N_CORES, 1, tiles * P).copy()

    batch_pad = np.full(npad, -1, dtype=np.int64)
    batch_pad[:N] = batch.astype(np.int64)
    g0 = np.zeros(N_CORES, dtype=np.int64)
    batchoff = np.full((N_CORES, P, tiles), float(P), dtype=np.float32)
    for k in range(N_CORES):
        b = batch_pad[k * rows_per_core:(k + 1) * rows_per_core]
        real = b >= 0
        assert real.any()
        g0[k] = b[real].min()
        span = int(b[real].max() - g0[k]) + 1
        assert span <= P - 1, f"graph span {span} exceeds pooling tile"
        off = np.full(rows_per_core, float(P), dtype=np.float32)
        off[real] = (b[real] - g0[k]).astype(np.float32)
        batchoff[k] = off.reshape(tiles, P).T

    # j-major iota const: iota_jm[p, j*(c_max+1)+c] = j. The S build runs
    # j-outer/c-inner so every operand's innermost AP dim is unit-stride,
    # which is what unlocks the DVE 2x 16-bit mode.
    iota_jm = np.tile(np.repeat(np.arange(P, dtype=np.float32), c_max + 1),
                      (P, 1)).astype(ml_dtypes.bfloat16)
    # same j-major layout with a tile axis, for the batched pool-selector
    iota_pool = np.tile(np.repeat(np.arange(P, dtype=np.float32), tiles),
                        (P, 1)).astype(ml_dtypes.bfloat16)
    cnt_g = np.bincount(batch.astype(np.int64), minlength=G).astype(np.float32)

    return dict(N=N, npad=npad, rows_per_core=rows_per_core,
                tiles_per_core=tiles, C_lo=C_lo, C_hi=C_hi, c_max=c_max,
                sum_ca=sum_ca, L_lo=L_lo, L_hi=L_hi,
                lo_pos=lo_pos, hi_pos=hi_pos,
                idx_lo_sb=idx_lo_sb, idx_hi_sb=idx_hi_sb,
                dstoff=dstoff.astype(ml_dtypes.bfloat16),
                dinv_slab=dinv_slab, dinv2_slab=dinv2_slab,
                sdeg_rows=sdeg_rows, g0=g0,
                batchoff=batchoff.astype(ml_dtypes.bfloat16),
                iota_jm=iota_jm, iota_pool=iota_pool, xhat=xhat, cnt_g=cnt_g)


# ---------------------------------------------------------------- device

def build_layer(pre, last_layer: bool, reps: int = 1, bf16_table: bool = True,
                parts: str = "full"):
    """One SPMD program: grouped bulk gathers + SpMM + dense matmul per tile.
    last_layer=False: hhat = relu(dinv^2 * (zW) + dinv*b)  [= dinv*h1 table]
    last_layer=True:  h2 = dinv * zW + b, pooled[goff] += sum h2
    """
    tiles = pre['tiles_per_core']
    C_lo, C_hi = pre['C_lo'], pre['C_hi']
    c_max = pre['c_max']
    sum_ca = pre['sum_ca']
    L_lo, L_hi = pre['L_lo'], pre['L_hi']
    lo_pos, hi_pos = pre['lo_pos'], pre['hi_pos']
    npad = pre['npad']
    assert tiles % GT == 0
    sumC_lo, sumC_hi = int(C_lo.sum()), int(C_hi.sum())
    lo_wins = _windows(sumC_lo, GATHER_BUDGET)
    hi_wins = _windows(sumC_hi, GATHER_BUDGET)
    WCH = GATHER_BUDGET // P      # chunks per full window
    lo_cstart = np.concatenate([[0], np.cumsum(C_lo)])  # global lo chunk idx
    hi_cstart = np.concatenate([[0], np.cumsum(C_hi)])

    TDT = BF16 if bf16_table else F32
    nc = bacc.Bacc(get_trn_type() or "TRN2", target_bir_lowering=False,
                   debug=False, num_swdge_queues=N_SWDGE_Q)
    table_lo = nc.dram_tensor("table_lo", [SPLIT, D], TDT, kind="ExternalInput").ap()
    table_hi = nc.dram_tensor("table_hi", [npad - SPLIT, D], TDT,
                              kind="ExternalInput").ap()
    # slab-transposed [p, t*D] layout: per-partition contiguous group slices
    self_rows = nc.dram_tensor("self_rows", [P, tiles * D], TDT,
                               kind="ExternalInput").ap()
    idx_lo = nc.dram_tensor("idx_lo", [P, L_lo // 16], I16, kind="ExternalInput").ap()
    idx_hi = nc.dram_tensor("idx_hi", [P, L_hi // 16], I16, kind="ExternalInput").ap()
    dstoff = nc.dram_tensor("dstoff", [P, sum_ca], BF16, kind="ExternalInput").ap()
    iota_b = nc.dram_tensor("iota_b", [P, (c_max + 1) * D], BF16,
                            kind="ExternalInput").ap()
    if last_layer:
        iota_p = nc.dram_tensor("iota_p", [P, tiles * D], BF16,
                                kind="ExternalInput").ap()
    Wt = nc.dram_tensor("W", [D, D], F32, kind="ExternalInput").ap()
    bt = nc.dram_tensor("b", [1, D], F32, kind="ExternalInput").ap()
    dinv = nc.dram_tensor("dinv", [P, tiles], F32, kind="ExternalInput").ap()
    sdeg = nc.dram_tensor("sdeg", [1, tiles * P], F32, kind="ExternalInput").ap()
    if last_layer:
        batchoff = nc.dram_tensor("batchoff", [P, tiles], BF16,
                                  kind="ExternalInput").ap()
        pooled = nc.dram_tensor("pooled", [P, D], F32, kind="ExternalOutput").ap()
    else:
        hhat = nc.dram_tensor("hhat", [P, tiles * D], TDT, kind="ExternalOutput").ap()



    with tile.TileContext(nc) as tc:
        with tc.tile_pool(name="const", bufs=1) as cp, \
             tc.tile_pool(name="glo", bufs=6) as glp, \
             tc.tile_pool(name="ghi", bufs=6) as ghp, \
             tc.tile_pool(name="gself", bufs=2) as gsp, \
             tc.tile_pool(name="sel", bufs=3) as sp, \
             tc.tile_pool(name="small", bufs=3) as mp, \
             tc.tile_pool(name="hout", bufs=2) as hp, \
             tc.tile_pool(name="ps1", bufs=2, space="PSUM") as pp1, \
             tc.tile_pool(name="ps2", bufs=2, space="PSUM") as pp2, \
             tc.tile_pool(name="ps3", bufs=2, space="PSUM") as pp3:
            ilo_t = cp.tile([P, L_lo // 16], I16)
            ihi_t = cp.tile([P, L_hi // 16], I16)
            dst_t = cp.tile([P, sum_ca], BF16)
            iob_t = cp.tile([P, (c_max + 1) * D], BF16)
            W_t = cp.tile([D, D], F32)
            b_t = cp.tile([1, D], F32)
            sdeg_t = cp.tile([1, tiles * P], F32)
            dinv_t = cp.tile([P, tiles], F32)
            nc.sync.dma_start(out=ilo_t[:], in_=idx_lo[:])
            nc.sync.dma_start(out=ihi_t[:], in_=idx_hi[:])
            nc.sync.dma_start(out=dst_t[:], in_=dstoff[:])
            nc.sync.dma_start(out=iob_t[:], in_=iota_b[:])
            nc.sync.dma_start(out=W_t[:], in_=Wt[:])
            nc.sync.dma_start(out=b_t[:], in_=bt[:])
            nc.sync.dma_start(out=sdeg_t[:], in_=sdeg[:])
            nc.sync.dma_start(out=dinv_t[:], in_=dinv[:])
            if last_layer:
                boff_t = cp.tile([P, tiles], BF16)
                iop_t = cp.tile([P, tiles * D], BF16)
                nc.sync.dma_start(out=boff_t[:], in_=batchoff[:])
                nc.sync.dma_start(out=iop_t[:], in_=iota_p[:])
                # all 49 pool one-hot selectors in one 2x-mode DVE op
                ptall = cp.tile([P, tiles * D], BF16)
                ptall_cm = ptall[:].rearrange("p (j t) -> p t j", t=tiles)
                nc.vector.tensor_tensor(
                    out=ptall[:].rearrange("p (j t) -> p j t", t=tiles),
                    in0=boff_t[:].rearrange("p (t o) -> p o t",
                                            o=1).to_broadcast([P, D, tiles]),
                    in1=iop_t[:].rearrange("p (j t) -> p j t", t=tiles),
                    op=mybir.AluOpType.is_equal)

            dummy_g = None
            if parts == "compute":
                dummy_g = cp.tile([P, WCH * D], TDT)
                nc.vector.memset(dummy_g[:], 0.0)

            for rep in range(reps):
                if last_layer:
                    pool_acc = pp3.tile([P, D], F32, space="PSUM", tag="pacc")
                cola = 0
                gself = hh = None
                gq = 0
                lo_bufs, hi_bufs = {}, {}
                lo_next = hi_next = 0
                for t in range(tiles):
                    # issue gather windows covering this tile's chunks
                    while (lo_next < len(lo_wins)
                           and lo_wins[lo_next][0] < lo_cstart[t + 1]):
                        c0, nch = lo_wins[lo_next]
                        buf = (dummy_g if parts == "compute"
                               else glp.tile([P, WCH * D], TDT, tag="glo"))
                        if parts != "compute":
                            nc.gpsimd.dma_gather(
                                buf[:, :nch * D].rearrange("p (c j) -> p c j", j=D),
                                table_lo[:],
                                ilo_t[:, c0 * P // 16:(c0 + nch) * P // 16],
                                nch * P, nch * P, D, single_packet=SINGLE_PACKET,
                                queue_num=gq % N_SWDGE_Q)
                        gq += 1
                        lo_bufs[lo_next] = buf
                        lo_next += 1
                    while (hi_next < len(hi_wins)
                           and hi_wins[hi_next][0] < hi_cstart[t + 1]):
                        c0, nch = hi_wins[hi_next]
                        buf = (dummy_g if parts == "compute"
                               else ghp.tile([P, WCH * D], TDT, tag="ghi"))
                        if parts != "compute":
                            nc.gpsimd.dma_gather(
                                buf[:, :nch * D].rearrange("p (c j) -> p c j", j=D),
                                table_hi[:],
                                ihi_t[:, c0 * P // 16:(c0 + nch) * P // 16],
                                nch * P, nch * P, D, single_packet=SINGLE_PACKET,
                                queue_num=gq % N_SWDGE_Q)
                        gq += 1
                        hi_bufs[hi_next] = buf
                        hi_next += 1
                    if parts == "gather":
                        cola += int(C_lo[t]) + int(C_hi[t]) + 1
                        continue
                    if t % GT == 0:
                        t0 = t
                        gself = gsp.tile([P, GT * D], TDT, tag="gs")
                        nc.sync.dma_start(
                            out=gself[:],
                            in_=self_rows[:, t0 * D:(t0 + GT) * D])
                        if not last_layer:
                            hh = hp.tile([P, GT * D], TDT, tag="hh")
                    Cl, Ch = int(C_lo[t]), int(C_hi[t])
                    C = Cl + Ch
                    # S built j-major (S[p, j, c]) so every operand's innermost
                    # AP dim is unit-stride -> DVE 2x 16-bit mode. The matmul
                    # then reads chunk c as a stride-(C+1) rhs slice.
                    S = sp.tile([P, (c_max + 1) * D], BF16, tag="s")
                    nc.vector.tensor_tensor(
                        out=S[:, :(C + 1) * D].rearrange("p (j c) -> p j c",
                                                         c=C + 1),
                        in0=dst_t[:, cola:cola + C + 1].rearrange(
                            "p (c o) -> p o c", o=1).to_broadcast([P, D, C + 1]),
                        in1=iob_t[:].rearrange("p (j c) -> p j c",
                                               c=c_max + 1)[:, :, :C + 1],
                        op=mybir.AluOpType.is_equal)
                    S_cm = S[:, :(C + 1) * D].rearrange("p (j c) -> p c j",
                                                        c=C + 1)
                    psumT = pp1.tile([P, D], F32, space="PSUM", tag="pT")
                    for c in range(C + 1):
                        if c < Cl:
                            q = int(lo_cstart[t]) + c
                            lhsT = lo_bufs[q // WCH][:, (q % WCH) * D:
                                                     (q % WCH + 1) * D]
                        elif c < C:
                            q = int(hi_cstart[t]) + (c - Cl)
                            lhsT = hi_bufs[q // WCH][:, (q % WCH) * D:
                                                     (q % WCH + 1) * D]
                        else:
                            lhsT = gself[:, (t - t0) * D:(t - t0 + 1) * D]
                        nc.tensor.matmul(out=psumT[:], lhsT=lhsT,
                                         rhs=S_cm[:, c],
                                         start=(c == 0), stop=(c == C))
                    lhs_sb = mp.tile([P, D], F32, tag="lhs")
                    nc.scalar.activation(out=lhs_sb[:], in_=psumT[:],
                                         func=mybir.ActivationFunctionType.Copy)
                    psum2 = pp2.tile([P, D], F32, space="PSUM", tag="p2")
                    nc.tensor.matmul(out=psum2[:], lhsT=lhs_sb[:], rhs=W_t[:],
                                     start=True, stop=False)
                    nc.tensor.matmul(out=psum2[:],
                                     lhsT=sdeg_t[:, t * P:(t + 1) * P],
                                     rhs=b_t[:], start=False, stop=True)
                    if last_layer:
                        out_sb = mp.tile([P, D], BF16, tag="out")
                        nc.scalar.activation(out=out_sb[:], in_=psum2[:],
                                             func=mybir.ActivationFunctionType.Copy,
                                             scale=dinv_t[:, t:t + 1])
                        nc.tensor.matmul(out=pool_acc[:], lhsT=ptall_cm[:, t],
                                         rhs=out_sb[:],
                                         start=(t == 0), stop=(t == tiles - 1))
                    else:
                        # hh = relu(dinv^2 * zW + dinv*b) = dinv * h1
                        nc.scalar.activation(out=hh[:, (t - t0) * D:(t - t0 + 1) * D],
                                             in_=psum2[:],
                                             func=mybir.ActivationFunctionType.Relu,
                                             scale=dinv_t[:, t:t + 1])
                    cola += C + 1
                    if not last_layer and t % GT == GT - 1:
                        nc.sync.dma_start(
                            out=hhat[:, t0 * D:(t0 + GT) * D],
                            in_=hh[:])
                if last_layer and parts != "gather":
                    pooled_sb = mp.tile([P, D], F32, tag="pool_out")
                    nc.scalar.activation(out=pooled_sb[:], in_=pool_acc[:],
                                         func=mybir.ActivationFunctionType.Copy)
                    nc.sync.dma_start(out=pooled[:], in_=pooled_sb[:])
    nc.compile()
    return nc


def _slab_T(slab, tiles):
    # [tiles*P, D] row-major -> [P, tiles*D] per-partition-contiguous
    return np.ascontiguousarray(
        slab.reshape(tiles, P, D).transpose(1, 0, 2).reshape(P, tiles * D))


def _in_maps(pre, table_np, W, b, last_layer):
    maps = []
    rpc = pre['rows_per_core']
    tiles = pre['tiles_per_core']
    t_lo = np.ascontiguousarray(table_np[:SPLIT])
    t_hi = np.ascontiguousarray(table_np[SPLIT:])
    for k in range(N_CORES):
        m = dict(table_lo=t_lo, table_hi=t_hi,
                 self_rows=_slab_T(table_np[k * rpc:(k + 1) * rpc], tiles),
                 idx_lo=pre['idx_lo_sb'][k],
                 idx_hi=pre['idx_hi_sb'][k],
                 dstoff=pre['dstoff'][k],
                 iota_b=pre['iota_jm'],
                 **({'iota_p': pre['iota_pool']} if last_layer else {}),
                 W=np.ascontiguousarray(W, dtype=np.float32),
                 b=np.ascontiguousarray(b, dtype=np.float32).reshape(1, D),
                 dinv=pre['dinv_slab'][k] if last_layer else pre['dinv2_slab'][k],
                 sdeg=pre['sdeg_rows'][k])
        if last_layer:
            m['batchoff'] = pre['batchoff'][k]
        maps.append(m)
    return maps


def kernel(x, edge_index, batch, W1, b1, W2, b2):
    x = np.asarray(x); edge_index = np.asarray(edge_index)
    batch = np.asarray(batch)
    W1 = np.asarray(W1); b1 = np.asarray(b1)
    W2 = np.asarray(W2); b2 = np.asarray(b2)

    pre = preprocess(x, edge_index, batch)
    core_ids = list(range(N_CORES))

    tdt = ml_dtypes.bfloat16 if USE_BF16 else np.float32
    table1 = pre['xhat'].astype(tdt)
    nc1 = build_layer(pre, last_layer=False, bf16_table=USE_BF16)
    res1 = run_bass_kernel_spmd(nc1, _in_maps(pre, table1, W1, b1, False),
                                core_ids).results

    h1hat = np.zeros((pre['npad'], D), dtype=tdt)
    rpc = pre['rows_per_core']
    tiles = pre['tiles_per_core']
    for k in range(N_CORES):
        h1hat[k * rpc:(k + 1) * rpc] = (
            res1[k]['hhat'].reshape(P, tiles, D).transpose(1, 0, 2)
            .reshape(rpc, D))

    nc2 = build_layer(pre, last_layer=True, bf16_table=USE_BF16)
    res2 = run_bass_kernel_spmd(nc2, _in_maps(pre, h1hat, W2, b2, True),
                                core_ids).results

    pooled = np.zeros((G, D), dtype=np.float32)
    for k in range(N_CORES):
        part = res2[k]['pooled']
        g0 = int(pre['g0'][k])
        span = min(P, G - g0)
        pooled[g0:g0 + span] += part[:span]
    return pooled / np.maximum(pre['cnt_g'], 1.0)[:, None]


# revision 62
# speedup vs baseline: 6.4652x; 1.0667x over previous
"""Trainium2 Bass kernel for a 2-layer GCN encoder (GCNConv x2 + global mean pool).

Math: with A' = A + I and deg = indegree(A') (symmetric-norm GCN),
    gcn(h, W, b) = D^-1/2 A' D^-1/2 (h) W + b
factorized as  out = dinv * (A' @ (dinv * h)) @ W + b   (dinv = deg^-1/2)
so the SpMM is pure 0/1 structure; per-edge norms become per-node row scales.

Sharding: dst-node ranges across 8 cores (6272 padded rows each). Each core:
  - bulk-gathers source rows of the (replicated) scaled feature table from HBM
    with InstDMAGatherAnt (one SWDGE op per ~7-tile group instead of one
    indirect op per 128 rows: the ~1us/op descriptor-generation fixed cost
    dominated the old kernel). Indices are int16, so the table is split at row
    32768 into lo/hi halves with one gather each per group,
  - streams each tile's self-loop block with one regular HWDGE DMA per group,
  - builds the 0/1 scatter matrix on VectorE in bf16 (compare dst-offset vs
    iota; the self-loop chunk's column is an identity pattern),
  - scatter-adds via TensorE one-hot matmuls accumulating in PSUM (psum_T is
    [din, dstoff], already transposed for the dense W matmul),
  - dense W matmul + rank-1 bias matmul; ScalarE applies relu and the row
    scale (layer 1 uses scale=dinv^2, folding both dinv factors around the
    relu: dinv*relu(dinv*zW + b) == relu(dinv^2*zW*... ) since dinv > 0).
Two SPMD launches (layer 1 -> host allgather of the 1.6MB/core slabs ->
layer 2 + graph pooling via one-hot matmul over sorted batch ids).
"""
import math
import numpy as np
import ml_dtypes

from concourse import bass, mybir, tile, bacc
from concourse.bass_utils import run_bass_kernel_spmd
from concourse._compat import get_trn_type

N_CORES = 8
P = 128          # partitions / tile rows
D = 128          # feature dim
G = 512          # number of graphs (fixed by the problem)
SPLIT = 32768    # dma_gather int16 index limit: table split row
GT = 7           # dst tiles per self-rows DMA group (49 = 7 x 7)
GATHER_BUDGET = 896   # 56+1 descs/lane: under the 64-desc packet ceiling
SINGLE_PACKET = True  # one packet per lane per op drains far faster
N_SWDGE_Q = 4         # round-robin gathers over SWDGE queues: each SDMA
                      # engine interleaves queue contexts, ~4x gather rate
F32 = mybir.dt.float32
BF16 = mybir.dt.bfloat16
I16 = mybir.dt.int16

USE_BF16 = True     # bf16 gather table (half the random-gather bytes)


def _windows(total_chunks, budget_slots):
    """Slice a chunk stream into fixed-size gather windows (last one ragged).
    Windows are tile-agnostic: a tile's chunks may span two windows."""
    W = budget_slots // P
    return [(w * W, min(W, total_chunks - w * W))
            for w in range((total_chunks + W - 1) // W)]


# ---------------------------------------------------------------- host prep

def preprocess(x, edge_index, batch):
    N = x.shape[0]
    rows_per_core = int(math.ceil(N / (N_CORES * P))) * P
    npad = rows_per_core * N_CORES
    tiles = rows_per_core // P
    n_tiles_g = N_CORES * tiles

    src = edge_index[0].astype(np.int64)
    dst = edge_index[1].astype(np.int64)
    # Self-loops are handled as a per-tile sequential-DMA chunk (no gather
    # slots); only real edges go through the sorted streams.
    deg = (np.bincount(dst, minlength=N) + 1).astype(np.float32)
    dinv = 1.0 / np.sqrt(np.maximum(deg, 1.0))

    xhat = np.zeros((npad, D), dtype=np.float32)
    xhat[:N] = x.astype(np.float32) * dinv[:, None]

    order = np.argsort(dst, kind="stable")
    src_s = src[order].astype(np.int64)
    dst_s = dst[order]
    bounds = np.searchsorted(dst_s, np.arange(0, npad + 1, P))

    # per-global-tile lo/hi edge partition (src < SPLIT goes to the lo gather)
    lo_srcs, lo_offs, hi_srcs, hi_offs = [], [], [], []
    n_lo = np.zeros(n_tiles_g, dtype=np.int64)
    n_hi = np.zeros(n_tiles_g, dtype=np.int64)
    for gt in range(n_tiles_g):
        s, e = int(bounds[gt]), int(bounds[gt + 1])
        seg_src = src_s[s:e]
        seg_off = (dst_s[s:e] - gt * P).astype(np.float32)
        m = seg_src < SPLIT
        lo_srcs.append(seg_src[m]); lo_offs.append(seg_off[m])
        hi_srcs.append(seg_src[~m] - SPLIT); hi_offs.append(seg_off[~m])
        n_lo[gt] = int(m.sum()); n_hi[gt] = int((~m).sum())

    C_lo = np.ceil(n_lo.reshape(N_CORES, tiles).max(axis=0) / P).astype(np.int64)
    C_hi = np.ceil(n_hi.reshape(N_CORES, tiles).max(axis=0) / P).astype(np.int64)
    c_tot = C_lo + C_hi                      # chunks per tile w/o self chunk
    c_max = int(c_tot.max())
    sum_ca = int(c_tot.sum()) + tiles        # dstoff cols incl self chunk

    L_lo = int(C_lo.sum()) * P               # lo gather slots per core
    L_hi = int(C_hi.sum()) * P
    lo_pos = np.concatenate([[0], np.cumsum(C_lo * P)])
    hi_pos = np.concatenate([[0], np.cumsum(C_hi * P)])

    idx_lo = np.zeros((N_CORES, L_lo), dtype=np.int32)
    idx_hi = np.zeros((N_CORES, L_hi), dtype=np.int32)
    dstoff = np.full((N_CORES, P, sum_ca), float(P), dtype=np.float32)
    for k in range(N_CORES):
        cola = 0
        for t in range(tiles):
            gt = k * tiles + t
            Cl, Ch = int(C_lo[t]), int(C_hi[t])
            nl, nh = int(n_lo[gt]), int(n_hi[gt])
            if nl:
                idx_lo[k, lo_pos[t]:lo_pos[t] + nl] = lo_srcs[gt]
                bo = np.full(Cl * P, float(P), dtype=np.float32)
                bo[:nl] = lo_offs[gt]
                dstoff[k, :, cola:cola + Cl] = bo.reshape(Cl, P).T
            cola += Cl
            if nh:
                idx_hi[k, hi_pos[t]:hi_pos[t] + nh] = hi_srcs[gt]
                bo = np.full(Ch * P, float(P), dtype=np.float32)
                bo[:nh] = hi_offs[gt]
                dstoff[k, :, cola:cola + Ch] = bo.reshape(Ch, P).T
            cola += Ch
            # identity column: self-loop chunk scatters partition e -> column e
            # (pad rows contribute zero because xhat is zero there)
            dstoff[k, :, cola] = np.arange(P, dtype=np.float32)
            cola += 1
    assert cola == sum_ca

    def idx_sbuf(a):
        # dma_gather idx layout: element i at partition i%16, col i//16,
        # replicated across all eight 16-partition stripes.
        sb = a.reshape(-1, 16).T
        return np.ascontiguousarray(np.tile(sb, (8, 1)).astype(np.int16))

    idx_lo_sb = np.stack([idx_sbuf(idx_lo[k]) for k in range(N_CORES)])
    idx_hi_sb = np.stack([idx_sbuf(idx_hi[k]) for k in range(N_CORES)])

    dinv_pad = np.zeros(npad, dtype=np.float32)
    dinv_pad[:N] = dinv
    dinv_slab = dinv_pad.reshape(N_CORES, tiles, P).transpose(0, 2, 1).copy()
    dinv2_slab = (dinv_slab * dinv_slab).copy()
    # sqrt(deg) per row (0 on pad rows): the bias is injected into PSUM as
    # sdeg[p]*b[j]; the later per-row scale turns it into the exact bias.
    sdeg_pad = np.zeros(npad, dtype=np.float32)
    sdeg_pad[:N] = np.sqrt(np.maximum(deg, 1.0))
    sdeg_rows = sdeg_pad.reshape(N_CORES, 1, tiles * P).copy()

    batch_pad = np.full(npad, -1, dtype=np.int64)
    batch_pad[:N] = batch.astype(np.int64)
    g0 = np.zeros(N_CORES, dtype=np.int64)
    batchoff = np.full((N_CORES, P, tiles), float(P), dtype=np.float32)
    for k in range(N_CORES):
        b = batch_pad[k * rows_per_core:(k + 1) * rows_per_core]
        real = b >= 0
        assert real.any()
        g0[k] = b[real].min()
        span = int(b[real].max() - g0[k]) + 1
        assert span <= P - 1, f"graph span {span} exceeds pooling tile"
        off = np.full(rows_per_core, float(P), dtype=np.float32)
        off[real] = (b[real] - g0[k]).astype(np.float32)
        batchoff[k] = off.reshape(tiles, P).T

    # j-major iota const: iota_jm[p, j*(c_max+1)+c] = j. The S build runs
    # j-outer/c-inner so every operand's innermost AP dim is unit-stride,
    # which is what unlocks the DVE 2x 16-bit mode.
    iota_jm = np.tile(np.repeat(np.arange(P, dtype=np.float32), c_max + 1),
                      (P, 1)).astype(ml_dtypes.bfloat16)
    # same j-major layout with a tile axis, for the batched pool-selector
    iota_pool = np.tile(np.repeat(np.arange(P, dtype=np.float32), tiles),
                        (P, 1)).astype(ml_dtypes.bfloat16)
    cnt_g = np.bincount(batch.astype(np.int64), minlength=G).astype(np.float32)

    return dict(N=N, npad=npad, rows_per_core=rows_per_core,
                tiles_per_core=tiles, C_lo=C_lo, C_hi=C_hi, c_max=c_max,
                sum_ca=sum_ca, L_lo=L_lo, L_hi=L_hi,
                lo_pos=lo_pos, hi_pos=hi_pos,
                idx_lo_sb=idx_lo_sb, idx_hi_sb=idx_hi_sb,
                dstoff=dstoff.astype(ml_dtypes.bfloat16),
                dinv_slab=dinv_slab, dinv2_slab=dinv2_slab,
                sdeg_rows=sdeg_rows, g0=g0,
                batchoff=batchoff.astype(ml_dtypes.bfloat16),
                iota_jm=iota_jm, iota_pool=iota_pool, xhat=xhat, cnt_g=cnt_g)


# ---------------------------------------------------------------- device

def build_layer(pre, last_layer: bool, reps: int = 1, bf16_table: bool = True,
                parts: str = "full"):
    """One SPMD program: grouped bulk gathers + SpMM + dense matmul per tile.
    last_layer=False: hhat = relu(dinv^2 * (zW) + dinv*b)  [= dinv*h1 table]
    last_layer=True:  h2 = dinv * zW + b, pooled[goff] += sum h2
    """
    tiles = pre['tiles_per_core']
    C_lo, C_hi = pre['C_lo'], pre['C_hi']
    c_max = pre['c_max']
    sum_ca = pre['sum_ca']
    L_lo, L_hi = pre['L_lo'], pre['L_hi']
    lo_pos, hi_pos = pre['lo_pos'], pre['hi_pos']
    npad = pre['npad']
    assert tiles % GT == 0
    sumC_lo, sumC_hi = int(C_lo.sum()), int(C_hi.sum())
    lo_wins = _windows(sumC_lo, GATHER_BUDGET)
    hi_wins = _windows(sumC_hi, GATHER_BUDGET)
    WCH = GATHER_BUDGET // P      # chunks per full window
    lo_cstart = np.concatenate([[0], np.cumsum(C_lo)])  # global lo chunk idx
    hi_cstart = np.concatenate([[0], np.cumsum(C_hi)])

    TDT = BF16 if bf16_table else F32
    nc = bacc.Bacc(get_trn_type() or "TRN2", target_bir_lowering=False,
                   debug=False, num_swdge_queues=N_SWDGE_Q)
    table_lo = nc.dram_tensor("table_lo", [SPLIT, D], TDT, kind="ExternalInput").ap()
    table_hi = nc.dram_tensor("table_hi", [npad - SPLIT, D], TDT,
                              kind="ExternalInput").ap()
    # slab-transposed [p, t*D] layout: per-partition contiguous group slices
    self_rows = nc.dram_tensor("self_rows", [P, tiles * D], TDT,
                               kind="ExternalInput").ap()
    idx_lo = nc.dram_tensor("idx_lo", [P, L_lo // 16], I16, kind="ExternalInput").ap()
    idx_hi = nc.dram_tensor("idx_hi", [P, L_hi // 16], I16, kind="ExternalInput").ap()
    dstoff = nc.dram_tensor("dstoff", [P, sum_ca], BF16, kind="ExternalInput").ap()
    iota_b = nc.dram_tensor("iota_b", [P, (c_max + 1) * D], BF16,
                            kind="ExternalInput").ap()
    if last_layer:
        iota_p = nc.dram_tensor("iota_p", [P, tiles * D], BF16,
                                kind="ExternalInput").ap()
    Wt = nc.dram_tensor("W", [D, D], F32, kind="ExternalInput").ap()
    bt = nc.dram_tensor("b", [1, D], F32, kind="ExternalInput").ap()
    dinv = nc.dram_tensor("dinv", [P, tiles], F32, kind="ExternalInput").ap()
    sdeg = nc.dram_tensor("sdeg", [1, tiles * P], F32, kind="ExternalInput").ap()
    if last_layer:
        batchoff = nc.dram_tensor("batchoff", [P, tiles], BF16,
                                  kind="ExternalInput").ap()
        pooled = nc.dram_tensor("pooled", [P, D], F32, kind="ExternalOutput").ap()
    else:
        hhat = nc.dram_tensor("hhat", [P, tiles * D], TDT, kind="ExternalOutput").ap()



    with tile.TileContext(nc) as tc:
        with tc.tile_pool(name="const", bufs=1) as cp, \
             tc.tile_pool(name="glo", bufs=9) as glp, \
             tc.tile_pool(name="ghi", bufs=9) as ghp, \
             tc.tile_pool(name="gself", bufs=3) as gsp, \
             tc.tile_pool(name="sel", bufs=3) as sp, \
             tc.tile_pool(name="small", bufs=3) as mp, \
             tc.tile_pool(name="hout", bufs=2) as hp, \
             tc.tile_pool(name="ps1", bufs=2, space="PSUM") as pp1, \
             tc.tile_pool(name="ps2", bufs=2, space="PSUM") as pp2, \
             tc.tile_pool(name="ps3", bufs=2, space="PSUM") as pp3:
            ilo_t = cp.tile([P, L_lo // 16], I16)
            ihi_t = cp.tile([P, L_hi // 16], I16)
            dst_t = cp.tile([P, sum_ca], BF16)
            iob_t = cp.tile([P, (c_max + 1) * D], BF16)
            W_t = cp.tile([D, D], F32)
            b_t = cp.tile([1, D], F32)
            sdeg_t = cp.tile([1, tiles * P], F32)
            dinv_t = cp.tile([P, tiles], F32)
            nc.sync.dma_start(out=ilo_t[:], in_=idx_lo[:])
            nc.sync.dma_start(out=ihi_t[:], in_=idx_hi[:])
            nc.sync.dma_start(out=dst_t[:], in_=dstoff[:])
            nc.sync.dma_start(out=iob_t[:], in_=iota_b[:])
            nc.sync.dma_start(out=W_t[:], in_=Wt[:])
            nc.sync.dma_start(out=b_t[:], in_=bt[:])
            nc.sync.dma_start(out=sdeg_t[:], in_=sdeg[:])
            nc.sync.dma_start(out=dinv_t[:], in_=dinv[:])
            if last_layer:
                boff_t = cp.tile([P, tiles], BF16)
                iop_t = cp.tile([P, tiles * D], BF16)
                nc.sync.dma_start(out=boff_t[:], in_=batchoff[:])
                nc.sync.dma_start(out=iop_t[:], in_=iota_p[:])
                # all 49 pool one-hot selectors in one 2x-mode DVE op
                ptall = cp.tile([P, tiles * D], BF16)
                ptall_cm = ptall[:].rearrange("p (j t) -> p t j", t=tiles)
                nc.vector.tensor_tensor(
                    out=ptall[:].rearrange("p (j t) -> p j t", t=tiles),
                    in0=boff_t[:].rearrange("p (t o) -> p o t",
                                            o=1).to_broadcast([P, D, tiles]),
                    in1=iop_t[:].rearrange("p (j t) -> p j t", t=tiles),
                    op=mybir.AluOpType.is_equal)

            dummy_g = None
            if parts == "compute":
                dummy_g = cp.tile([P, WCH * D], TDT)
                nc.vector.memset(dummy_g[:], 0.0)

            for rep in range(reps):
                if last_layer:
                    pool_acc = pp3.tile([P, D], F32, space="PSUM", tag="pacc")
                cola = 0
                gself = hh = None
                gq = 0
                lo_bufs, hi_bufs = {}, {}
                lo_next = hi_next = 0
                for t in range(tiles):
                    # issue gather windows covering this tile's chunks, plus
                    # two tiles of lookahead so gathers run ahead of compute
                    ta = min(t + 3, tiles)
                    while (lo_next < len(lo_wins)
                           and lo_wins[lo_next][0] < lo_cstart[ta]):
                        c0, nch = lo_wins[lo_next]
                        buf = (dummy_g if parts == "compute"
                               else glp.tile([P, WCH * D], TDT, tag="glo"))
                        if parts != "compute":
                            nc.gpsimd.dma_gather(
                                buf[:, :nch * D].rearrange("p (c j) -> p c j", j=D),
                                table_lo[:],
                                ilo_t[:, c0 * P // 16:(c0 + nch) * P // 16],
                                nch * P, nch * P, D, single_packet=SINGLE_PACKET,
                                queue_num=gq % N_SWDGE_Q)
                        gq += 1
                        lo_bufs[lo_next] = buf
                        lo_next += 1
                    while (hi_next < len(hi_wins)
                           and hi_wins[hi_next][0] < hi_cstart[ta]):
                        c0, nch = hi_wins[hi_next]
                        buf = (dummy_g if parts == "compute"
                               else ghp.tile([P, WCH * D], TDT, tag="ghi"))
                        if parts != "compute":
                            nc.gpsimd.dma_gather(
                                buf[:, :nch * D].rearrange("p (c j) -> p c j", j=D),
                                table_hi[:],
                                ihi_t[:, c0 * P // 16:(c0 + nch) * P // 16],
                                nch * P, nch * P, D, single_packet=SINGLE_PACKET,
                                queue_num=gq % N_SWDGE_Q)
                        gq += 1
                        hi_bufs[hi_next] = buf
                        hi_next += 1
                    if parts == "gather":
                        cola += int(C_lo[t]) + int(C_hi[t]) + 1
                        continue
                    if t % GT == 0:
                        t0 = t
                        gself = gsp.tile([P, GT * D], TDT, tag="gs")
                        nc.sync.dma_start(
                            out=gself[:],
                            in_=self_rows[:, t0 * D:(t0 + GT) * D])
                        if not last_layer:
                            hh = hp.tile([P, GT * D], TDT, tag="hh")
                    Cl, Ch = int(C_lo[t]), int(C_hi[t])
                    C = Cl + Ch
                    # S built j-major (S[p, j, c]) so every operand's innermost
                    # AP dim is unit-stride -> DVE 2x 16-bit mode. The matmul
                    # then reads chunk c as a stride-(C+1) rhs slice.
                    S = sp.tile([P, (c_max + 1) * D], BF16, tag="s")
                    nc.vector.tensor_tensor(
                        out=S[:, :(C + 1) * D].rearrange("p (j c) -> p j c",
                                                         c=C + 1),
                        in0=dst_t[:, cola:cola + C + 1].rearrange(
                            "p (c o) -> p o c", o=1).to_broadcast([P, D, C + 1]),
                        in1=iob_t[:].rearrange("p (j c) -> p j c",
                                               c=c_max + 1)[:, :, :C + 1],
                        op=mybir.AluOpType.is_equal)
                    S_cm = S[:, :(C + 1) * D].rearrange("p (j c) -> p c j",
                                                        c=C + 1)
                    psumT = pp1.tile([P, D], F32, space="PSUM", tag="pT")
                    for c in range(C + 1):
                        if c < Cl:
                            q = int(lo_cstart[t]) + c
                            lhsT = lo_bufs[q // WCH][:, (q % WCH) * D:
                                                     (q % WCH + 1) * D]
                        elif c < C:
                            q = int(hi_cstart[t]) + (c - Cl)
                            lhsT = hi_bufs[q // WCH][:, (q % WCH) * D:
                                                     (q % WCH + 1) * D]
                        else:
                            lhsT = gself[:, (t - t0) * D:(t - t0 + 1) * D]
                        nc.tensor.matmul(out=psumT[:], lhsT=lhsT,
                                         rhs=S_cm[:, c],
                                         start=(c == 0), stop=(c == C))
                    lhs_sb = mp.tile([P, D], F32, tag="lhs")
                    nc.scalar.activation(out=lhs_sb[:], in_=psumT[:],
                                         func=mybir.ActivationFunctionType.Copy)
                    psum2 = pp2.tile([P, D], F32, space="PSUM", tag="p2")
                    nc.tensor.matmul(out=psum2[:], lhsT=lhs_sb[:], rhs=W_t[:],
                                     start=True, stop=False)
                    nc.tensor.matmul(out=psum2[:],
                                     lhsT=sdeg_t[:, t * P:(t + 1) * P],
                                     rhs=b_t[:], start=False, stop=True)
                    if last_layer:
                        out_sb = mp.tile([P, D], BF16, tag="out")
                        nc.scalar.activation(out=out_sb[:], in_=psum2[:],
                                             func=mybir.ActivationFunctionType.Copy,
                                             scale=dinv_t[:, t:t + 1])
                        nc.tensor.matmul(out=pool_acc[:], lhsT=ptall_cm[:, t],
                                         rhs=out_sb[:],
                                         start=(t == 0), stop=(t == tiles - 1))
                    else:
                        # hh = relu(dinv^2 * zW + dinv*b) = dinv * h1
                        nc.scalar.activation(out=hh[:, (t - t0) * D:(t - t0 + 1) * D],
                                             in_=psum2[:],
                                             func=mybir.ActivationFunctionType.Relu,
                                             scale=dinv_t[:, t:t + 1])
                    cola += C + 1
                    if not last_layer and t % GT == GT - 1:
                        nc.sync.dma_start(
                            out=hhat[:, t0 * D:(t0 + GT) * D],
                            in_=hh[:])
                if last_layer and parts != "gather":
                    pooled_sb = mp.tile([P, D], F32, tag="pool_out")
                    nc.scalar.activation(out=pooled_sb[:], in_=pool_acc[:],
                                         func=mybir.ActivationFunctionType.Copy)
                    nc.sync.dma_start(out=pooled[:], in_=pooled_sb[:])
    nc.compile()
    return nc


def _slab_T(slab, tiles):
    # [tiles*P, D] row-major -> [P, tiles*D] per-partition-contiguous
    return np.ascontiguousarray(
        slab.reshape(tiles, P, D).transpose(1, 0, 2).reshape(P, tiles * D))


def _in_maps(pre, table_np, W, b, last_layer):
    maps = []
    rpc = pre['rows_per_core']
    tiles = pre['tiles_per_core']
    t_lo = np.ascontiguousarray(table_np[:SPLIT])
    t_hi = np.ascontiguousarray(table_np[SPLIT:])
    for k in range(N_CORES):
        m = dict(table_lo=t_lo, table_hi=t_hi,
                 self_rows=_slab_T(table_np[k * rpc:(k + 1) * rpc], tiles),
                 idx_lo=pre['idx_lo_sb'][k],
                 idx_hi=pre['idx_hi_sb'][k],
                 dstoff=pre['dstoff'][k],
                 iota_b=pre['iota_jm'],
                 **({'iota_p': pre['iota_pool']} if last_layer else {}),
                 W=np.ascontiguousarray(W, dtype=np.float32),
                 b=np.ascontiguousarray(b, dtype=np.float32).reshape(1, D),
                 dinv=pre['dinv_slab'][k] if last_layer else pre['dinv2_slab'][k],
                 sdeg=pre['sdeg_rows'][k])
        if last_layer:
            m['batchoff'] = pre['batchoff'][k]
        maps.append(m)
    return maps


def kernel(x, edge_index, batch, W1, b1, W2, b2):
    x = np.asarray(x); edge_index = np.asarray(edge_index)
    batch = np.asarray(batch)
    W1 = np.asarray(W1); b1 = np.asarray(b1)
    W2 = np.asarray(W2); b2 = np.asarray(b2)

    pre = preprocess(x, edge_index, batch)
    core_ids = list(range(N_CORES))

    tdt = ml_dtypes.bfloat16 if USE_BF16 else np.float32
    table1 = pre['xhat'].astype(tdt)
    nc1 = build_layer(pre, last_layer=False, bf16_table=USE_BF16)
    res1 = run_bass_kernel_spmd(nc1, _in_maps(pre, table1, W1, b1, False),
                                core_ids).results

    h1hat = np.zeros((pre['npad'], D), dtype=tdt)
    rpc = pre['rows_per_core']
    tiles = pre['tiles_per_core']
    for k in range(N_CORES):
        h1hat[k * rpc:(k + 1) * rpc] = (
            res1[k]['hhat'].reshape(P, tiles, D).transpose(1, 0, 2)
            .reshape(rpc, D))

    nc2 = build_layer(pre, last_layer=True, bf16_table=USE_BF16)
    res2 = run_bass_kernel_spmd(nc2, _in_maps(pre, h1hat, W2, b2, True),
                                core_ids).results

    pooled = np.zeros((G, D), dtype=np.float32)
    for k in range(N_CORES):
        part = res2[k]['pooled']
        g0 = int(pre['g0'][k])
        span = min(P, G - g0)
        pooled[g0:g0 + span] += part[:span]
    return pooled / np.maximum(pre['cnt_g'], 1.0)[:, None]


# revision 64
# speedup vs baseline: 6.5572x; 1.0142x over previous
"""Trainium2 Bass kernel for a 2-layer GCN encoder (GCNConv x2 + global mean pool).

Math: with A' = A + I and deg = indegree(A') (symmetric-norm GCN),
    gcn(h, W, b) = D^-1/2 A' D^-1/2 (h) W + b
factorized as  out = dinv * (A' @ (dinv * h)) @ W + b   (dinv = deg^-1/2)
so the SpMM is pure 0/1 structure; per-edge norms become per-node row scales.

Sharding: dst-node ranges across 8 cores (6272 padded rows each). Each core:
  - bulk-gathers source rows of the (replicated) scaled feature table from HBM
    with InstDMAGatherAnt (one SWDGE op per ~7-tile group instead of one
    indirect op per 128 rows: the ~1us/op descriptor-generation fixed cost
    dominated the old kernel). Indices are int16, so the table is split at row
    32768 into lo/hi halves with one gather each per group,
  - streams each tile's self-loop block with one regular HWDGE DMA per group,
  - builds the 0/1 scatter matrix on VectorE in bf16 (compare dst-offset vs
    iota; the self-loop chunk's column is an identity pattern),
  - scatter-adds via TensorE one-hot matmuls accumulating in PSUM (psum_T is
    [din, dstoff], already transposed for the dense W matmul),
  - dense W matmul + rank-1 bias matmul; ScalarE applies relu and the row
    scale (layer 1 uses scale=dinv^2, folding both dinv factors around the
    relu: dinv*relu(dinv*zW + b) == relu(dinv^2*zW*... ) since dinv > 0).
Two SPMD launches (layer 1 -> host allgather of the 1.6MB/core slabs ->
layer 2 + graph pooling via one-hot matmul over sorted batch ids).
"""
import math
import numpy as np
import ml_dtypes

from concourse import bass, mybir, tile, bacc
from concourse.bass_utils import run_bass_kernel_spmd
from concourse._compat import get_trn_type

N_CORES = 8
P = 128          # partitions / tile rows
D = 128          # feature dim
G = 512          # number of graphs (fixed by the problem)
SPLIT = 32768    # dma_gather int16 index limit: table split row
GT = 7           # dst tiles per self-rows DMA group (49 = 7 x 7)
GATHER_BUDGET = 896   # 56+1 descs/lane: under the 64-desc packet ceiling
SINGLE_PACKET = True  # one packet per lane per op drains far faster
N_SWDGE_Q = 4         # round-robin gathers over SWDGE queues: each SDMA
                      # engine interleaves queue contexts, ~4x gather rate
F32 = mybir.dt.float32
BF16 = mybir.dt.bfloat16
I16 = mybir.dt.int16

USE_BF16 = True     # bf16 gather table (half the random-gather bytes)


def _windows(total_chunks, budget_slots):
    """Slice a chunk stream into fixed-size gather windows (last one ragged).
    Windows are tile-agnostic: a tile's chunks may span two windows."""
    W = budget_slots // P
    return [(w * W, min(W, total_chunks - w * W))
            for w in range((total_chunks + W - 1) // W)]


# ---------------------------------------------------------------- host prep

def preprocess(x, edge_index, batch):
    N = x.shape[0]
    rows_per_core = int(math.ceil(N / (N_CORES * P))) * P
    npad = rows_per_core * N_CORES
    tiles = rows_per_core // P
    n_tiles_g = N_CORES * tiles

    src = edge_index[0].astype(np.int64)
    dst = edge_index[1].astype(np.int64)
    # Self-loops are handled as a per-tile sequential-DMA chunk (no gather
    # slots); only real edges go through the sorted streams.
    deg = (np.bincount(dst, minlength=N) + 1).astype(np.float32)
    dinv = 1.0 / np.sqrt(np.maximum(deg, 1.0))

    xhat = np.zeros((npad, D), dtype=np.float32)
    xhat[:N] = x.astype(np.float32) * dinv[:, None]

    order = np.argsort(dst, kind="stable")
    src_s = src[order].astype(np.int64)
    dst_s = dst[order]
    bounds = np.searchsorted(dst_s, np.arange(0, npad + 1, P))

    # per-global-tile lo/hi edge partition (src < SPLIT goes to the lo gather)
    lo_srcs, lo_offs, hi_srcs, hi_offs = [], [], [], []
    n_lo = np.zeros(n_tiles_g, dtype=np.int64)
    n_hi = np.zeros(n_tiles_g, dtype=np.int64)
    for gt in range(n_tiles_g):
        s, e = int(bounds[gt]), int(bounds[gt + 1])
        seg_src = src_s[s:e]
        seg_off = (dst_s[s:e] - gt * P).astype(np.float32)
        m = seg_src < SPLIT
        lo_srcs.append(seg_src[m]); lo_offs.append(seg_off[m])
        hi_srcs.append(seg_src[~m] - SPLIT); hi_offs.append(seg_off[~m])
        n_lo[gt] = int(m.sum()); n_hi[gt] = int((~m).sum())

    C_lo = np.ceil(n_lo.reshape(N_CORES, tiles).max(axis=0) / P).astype(np.int64)
    C_hi = np.ceil(n_hi.reshape(N_CORES, tiles).max(axis=0) / P).astype(np.int64)
    c_tot = C_lo + C_hi                      # chunks per tile w/o self chunk
    c_max = int(c_tot.max())
    sum_ca = int(c_tot.sum()) + tiles        # dstoff cols incl self chunk

    L_lo = int(C_lo.sum()) * P               # lo gather slots per core
    L_hi = int(C_hi.sum()) * P
    lo_pos = np.concatenate([[0], np.cumsum(C_lo * P)])
    hi_pos = np.concatenate([[0], np.cumsum(C_hi * P)])

    idx_lo = np.zeros((N_CORES, L_lo), dtype=np.int32)
    idx_hi = np.zeros((N_CORES, L_hi), dtype=np.int32)
    dstoff = np.full((N_CORES, P, sum_ca), float(P), dtype=np.float32)
    for k in range(N_CORES):
        cola = 0
        for t in range(tiles):
            gt = k * tiles + t
            Cl, Ch = int(C_lo[t]), int(C_hi[t])
            nl, nh = int(n_lo[gt]), int(n_hi[gt])
            if nl:
                idx_lo[k, lo_pos[t]:lo_pos[t] + nl] = lo_srcs[gt]
                bo = np.full(Cl * P, float(P), dtype=np.float32)
                bo[:nl] = lo_offs[gt]
                dstoff[k, :, cola:cola + Cl] = bo.reshape(Cl, P).T
            cola += Cl
            if nh:
                idx_hi[k, hi_pos[t]:hi_pos[t] + nh] = hi_srcs[gt]
                bo = np.full(Ch * P, float(P), dtype=np.float32)
                bo[:nh] = hi_offs[gt]
                dstoff[k, :, cola:cola + Ch] = bo.reshape(Ch, P).T
            cola += Ch
            # identity column: self-loop chunk scatters partition e -> column e
            # (pad rows contribute zero because xhat is zero there)
            dstoff[k, :, cola] = np.arange(P, dtype=np.float32)
            cola += 1
    assert cola == sum_ca

    def idx_sbuf(a):
        # dma_gather idx layout: element i at partition i%16, col i//16,
        # replicated across all eight 16-partition stripes.
        sb = a.reshape(-1, 16).T
        return np.ascontiguousarray(np.tile(sb, (8, 1)).astype(np.int16))

    idx_lo_sb = np.stack([idx_sbuf(idx_lo[k]) for k in range(N_CORES)])
    idx_hi_sb = np.stack([idx_sbuf(idx_hi[k]) for k in range(N_CORES)])

    dinv_pad = np.zeros(npad, dtype=np.float32)
    dinv_pad[:N] = dinv
    dinv_slab = dinv_pad.reshape(N_CORES, tiles, P).transpose(0, 2, 1).copy()
    dinv2_slab = (dinv_slab * dinv_slab).copy()
    # sqrt(deg) per row (0 on pad rows): the bias is injected into PSUM as
    # sdeg[p]*b[j]; the later per-row scale turns it into the exact bias.
    sdeg_pad = np.zeros(npad, dtype=np.float32)
    sdeg_pad[:N] = np.sqrt(np.maximum(deg, 1.0))
    sdeg_rows = sdeg_pad.reshape(N_CORES, 1, tiles * P).copy()

    batch_pad = np.full(npad, -1, dtype=np.int64)
    batch_pad[:N] = batch.astype(np.int64)
    g0 = np.zeros(N_CORES, dtype=np.int64)
    batchoff = np.full((N_CORES, P, tiles), float(P), dtype=np.float32)
    for k in range(N_CORES):
        b = batch_pad[k * rows_per_core:(k + 1) * rows_per_core]
        real = b >= 0
        assert real.any()
        g0[k] = b[real].min()
        span = int(b[real].max() - g0[k]) + 1
        assert span <= P - 1, f"graph span {span} exceeds pooling tile"
        off = np.full(rows_per_core, float(P), dtype=np.float32)
        off[real] = (b[real] - g0[k]).astype(np.float32)
        batchoff[k] = off.reshape(tiles, P).T

    # j-major iota const: iota_jm[p, j*(c_max+1)+c] = j. The S build runs
    # j-outer/c-inner so every operand's innermost AP dim is unit-stride,
    # which is what unlocks the DVE 2x 16-bit mode.
    iota_jm = np.tile(np.repeat(np.arange(P, dtype=np.float32), c_max + 1),
                      (P, 1)).astype(ml_dtypes.bfloat16)
    # same j-major layout with a tile axis, for the batched pool-selector
    iota_pool = np.tile(np.repeat(np.arange(P, dtype=np.float32), tiles),
                        (P, 1)).astype(ml_dtypes.bfloat16)
    cnt_g = np.bincount(batch.astype(np.int64), minlength=G).astype(np.float32)

    return dict(N=N, npad=npad, rows_per_core=rows_per_core,
                tiles_per_core=tiles, C_lo=C_lo, C_hi=C_hi, c_max=c_max,
                sum_ca=sum_ca, L_lo=L_lo, L_hi=L_hi,
                lo_pos=lo_pos, hi_pos=hi_pos,
                idx_lo_sb=idx_lo_sb, idx_hi_sb=idx_hi_sb,
                dstoff=dstoff.astype(ml_dtypes.bfloat16),
                dinv_slab=dinv_slab, dinv2_slab=dinv2_slab,
                sdeg_rows=sdeg_rows, g0=g0,
                batchoff=batchoff.astype(ml_dtypes.bfloat16),
                iota_jm=iota_jm, iota_pool=iota_pool, xhat=xhat, cnt_g=cnt_g)


# ---------------------------------------------------------------- device

def build_layer(pre, last_layer: bool, reps: int = 1, bf16_table: bool = True,
                parts: str = "full"):
    """One SPMD program: grouped bulk gathers + SpMM + dense matmul per tile.
    last_layer=False: hhat = relu(dinv^2 * (zW) + dinv*b)  [= dinv*h1 table]
    last_layer=True:  h2 = dinv * zW + b, pooled[goff] += sum h2
    """
    tiles = pre['tiles_per_core']
    C_lo, C_hi = pre['C_lo'], pre['C_hi']
    c_max = pre['c_max']
    sum_ca = pre['sum_ca']
    L_lo, L_hi = pre['L_lo'], pre['L_hi']
    lo_pos, hi_pos = pre['lo_pos'], pre['hi_pos']
    npad = pre['npad']
    assert tiles % GT == 0
    sumC_lo, sumC_hi = int(C_lo.sum()), int(C_hi.sum())
    lo_wins = _windows(sumC_lo, GATHER_BUDGET)
    hi_wins = _windows(sumC_hi, GATHER_BUDGET)
    WCH = GATHER_BUDGET // P      # chunks per full window
    lo_cstart = np.concatenate([[0], np.cumsum(C_lo)])  # global lo chunk idx
    hi_cstart = np.concatenate([[0], np.cumsum(C_hi)])

    TDT = BF16 if bf16_table else F32
    nc = bacc.Bacc(get_trn_type() or "TRN2", target_bir_lowering=False,
                   debug=False, num_swdge_queues=N_SWDGE_Q)
    table_lo = nc.dram_tensor("table_lo", [SPLIT, D], TDT, kind="ExternalInput").ap()
    table_hi = nc.dram_tensor("table_hi", [npad - SPLIT, D], TDT,
                              kind="ExternalInput").ap()
    # slab-transposed [p, t*D] layout: per-partition contiguous group slices
    self_rows = nc.dram_tensor("self_rows", [P, tiles * D], TDT,
                               kind="ExternalInput").ap()
    idx_lo = nc.dram_tensor("idx_lo", [P, L_lo // 16], I16, kind="ExternalInput").ap()
    idx_hi = nc.dram_tensor("idx_hi", [P, L_hi // 16], I16, kind="ExternalInput").ap()
    dstoff = nc.dram_tensor("dstoff", [P, sum_ca], BF16, kind="ExternalInput").ap()
    iota_b = nc.dram_tensor("iota_b", [P, (c_max + 1) * D], BF16,
                            kind="ExternalInput").ap()
    if last_layer:
        iota_p = nc.dram_tensor("iota_p", [P, tiles * D], BF16,
                                kind="ExternalInput").ap()
    Wt = nc.dram_tensor("W", [D, D], F32, kind="ExternalInput").ap()
    bt = nc.dram_tensor("b", [1, D], F32, kind="ExternalInput").ap()
    dinv = nc.dram_tensor("dinv", [P, tiles], F32, kind="ExternalInput").ap()
    sdeg = nc.dram_tensor("sdeg", [1, tiles * P], F32, kind="ExternalInput").ap()
    if last_layer:
        batchoff = nc.dram_tensor("batchoff", [P, tiles], BF16,
                                  kind="ExternalInput").ap()
        pooled = nc.dram_tensor("pooled", [P, D], F32, kind="ExternalOutput").ap()
    else:
        hhat = nc.dram_tensor("hhat", [P, tiles * D], TDT, kind="ExternalOutput").ap()



    with tile.TileContext(nc) as tc:
        with tc.tile_pool(name="const", bufs=1) as cp, \
             tc.tile_pool(name="glo", bufs=12) as glp, \
             tc.tile_pool(name="ghi", bufs=12) as ghp, \
             tc.tile_pool(name="gself", bufs=3) as gsp, \
             tc.tile_pool(name="sel", bufs=4) as sp, \
             tc.tile_pool(name="small", bufs=4) as mp, \
             tc.tile_pool(name="hout", bufs=2) as hp, \
             tc.tile_pool(name="ps1", bufs=2, space="PSUM") as pp1, \
             tc.tile_pool(name="ps2", bufs=2, space="PSUM") as pp2, \
             tc.tile_pool(name="ps3", bufs=2, space="PSUM") as pp3:
            ilo_t = cp.tile([P, L_lo // 16], I16)
            ihi_t = cp.tile([P, L_hi // 16], I16)
            dst_t = cp.tile([P, sum_ca], BF16)
            iob_t = cp.tile([P, (c_max + 1) * D], BF16)
            W_t = cp.tile([D, D], F32)
            b_t = cp.tile([1, D], F32)
            sdeg_t = cp.tile([1, tiles * P], F32)
            dinv_t = cp.tile([P, tiles], F32)
            nc.sync.dma_start(out=ilo_t[:], in_=idx_lo[:])
            nc.sync.dma_start(out=ihi_t[:], in_=idx_hi[:])
            nc.sync.dma_start(out=dst_t[:], in_=dstoff[:])
            nc.sync.dma_start(out=iob_t[:], in_=iota_b[:])
            nc.sync.dma_start(out=W_t[:], in_=Wt[:])
            nc.sync.dma_start(out=b_t[:], in_=bt[:])
            nc.sync.dma_start(out=sdeg_t[:], in_=sdeg[:])
            nc.sync.dma_start(out=dinv_t[:], in_=dinv[:])
            if last_layer:
                boff_t = cp.tile([P, tiles], BF16)
                iop_t = cp.tile([P, tiles * D], BF16)
                nc.sync.dma_start(out=boff_t[:], in_=batchoff[:])
                nc.sync.dma_start(out=iop_t[:], in_=iota_p[:])
                # all 49 pool one-hot selectors in one 2x-mode DVE op
                ptall = cp.tile([P, tiles * D], BF16)
                ptall_cm = ptall[:].rearrange("p (j t) -> p t j", t=tiles)
                nc.vector.tensor_tensor(
                    out=ptall[:].rearrange("p (j t) -> p j t", t=tiles),
                    in0=boff_t[:].rearrange("p (t o) -> p o t",
                                            o=1).to_broadcast([P, D, tiles]),
                    in1=iop_t[:].rearrange("p (j t) -> p j t", t=tiles),
                    op=mybir.AluOpType.is_equal)

            dummy_g = None
            if parts == "compute":
                dummy_g = cp.tile([P, WCH * D], TDT)
                nc.vector.memset(dummy_g[:], 0.0)

            for rep in range(reps):
                if last_layer:
                    pool_acc = pp3.tile([P, D], F32, space="PSUM", tag="pacc")
                cola = 0
                gself = hh = None
                gq = 0
                lo_bufs, hi_bufs = {}, {}
                lo_next = hi_next = 0
                for t in range(tiles):
                    # issue gather windows covering this tile's chunks, plus
                    # two tiles of lookahead so gathers run ahead of compute
                    ta = min(t + 5, tiles)
                    while (lo_next < len(lo_wins)
                           and lo_wins[lo_next][0] < lo_cstart[ta]):
                        c0, nch = lo_wins[lo_next]
                        buf = (dummy_g if parts == "compute"
                               else glp.tile([P, WCH * D], TDT, tag="glo"))
                        if parts != "compute":
                            nc.gpsimd.dma_gather(
                                buf[:, :nch * D].rearrange("p (c j) -> p c j", j=D),
                                table_lo[:],
                                ilo_t[:, c0 * P // 16:(c0 + nch) * P // 16],
                                nch * P, nch * P, D, single_packet=SINGLE_PACKET,
                                queue_num=gq % N_SWDGE_Q)
                        gq += 1
                        lo_bufs[lo_next] = buf
                        lo_next += 1
                    while (hi_next < len(hi_wins)
                           and hi_wins[hi_next][0] < hi_cstart[ta]):
                        c0, nch = hi_wins[hi_next]
                        buf = (dummy_g if parts == "compute"
                               else ghp.tile([P, WCH * D], TDT, tag="ghi"))
                        if parts != "compute":
                            nc.gpsimd.dma_gather(
                                buf[:, :nch * D].rearrange("p (c j) -> p c j", j=D),
                                table_hi[:],
                                ihi_t[:, c0 * P // 16:(c0 + nch) * P // 16],
                                nch * P, nch * P, D, single_packet=SINGLE_PACKET,
                                queue_num=gq % N_SWDGE_Q)
                        gq += 1
                        hi_bufs[hi_next] = buf
                        hi_next += 1
                    if parts == "gather":
                        cola += int(C_lo[t]) + int(C_hi[t]) + 1
                        continue
                    if t % GT == 0:
                        t0 = t
                        gself = gsp.tile([P, GT * D], TDT, tag="gs")
                        nc.sync.dma_start(
                            out=gself[:],
                            in_=self_rows[:, t0 * D:(t0 + GT) * D])
                        if not last_layer:
                            hh = hp.tile([P, GT * D], TDT, tag="hh")
                    Cl, Ch = int(C_lo[t]), int(C_hi[t])
                    C = Cl + Ch
                    # S built j-major (S[p, j, c]) so every operand's innermost
                    # AP dim is unit-stride -> DVE 2x 16-bit mode. The matmul
                    # then reads chunk c as a stride-(C+1) rhs slice.
                    S = sp.tile([P, (c_max + 1) * D], BF16, tag="s")
                    nc.vector.tensor_tensor(
                        out=S[:, :(C + 1) * D].rearrange("p (j c) -> p j c",
                                                         c=C + 1),
                        in0=dst_t[:, cola:cola + C + 1].rearrange(
                            "p (c o) -> p o c", o=1).to_broadcast([P, D, C + 1]),
                        in1=iob_t[:].rearrange("p (j c) -> p j c",
                                               c=c_max + 1)[:, :, :C + 1],
                        op=mybir.AluOpType.is_equal)
                    S_cm = S[:, :(C + 1) * D].rearrange("p (j c) -> p c j",
                                                        c=C + 1)
                    psumT = pp1.tile([P, D], F32, space="PSUM", tag="pT")
                    for c in range(C + 1):
                        if c < Cl:
                            q = int(lo_cstart[t]) + c
                            lhsT = lo_bufs[q // WCH][:, (q % WCH) * D:
                                                     (q % WCH + 1) * D]
                        elif c < C:
                            q = int(hi_cstart[t]) + (c - Cl)
                            lhsT = hi_bufs[q // WCH][:, (q % WCH) * D:
                                                     (q % WCH + 1) * D]
                        else:
                            lhsT = gself[:, (t - t0) * D:(t - t0 + 1) * D]
                        nc.tensor.matmul(out=psumT[:], lhsT=lhsT,
                                         rhs=S_cm[:, c],
                                         start=(c == 0), stop=(c == C))
                    lhs_sb = mp.tile([P, D], F32, tag="lhs")
                    nc.scalar.activation(out=lhs_sb[:], in_=psumT[:],
                                         func=mybir.ActivationFunctionType.Copy)
                    psum2 = pp2.tile([P, D], F32, space="PSUM", tag="p2")
                    nc.tensor.matmul(out=psum2[:], lhsT=lhs_sb[:], rhs=W_t[:],
                                     start=True, stop=False)
                    nc.tensor.matmul(out=psum2[:],
                                     lhsT=sdeg_t[:, t * P:(t + 1) * P],
                                     rhs=b_t[:], start=False, stop=True)
                    if last_layer:
                        out_sb = mp.tile([P, D], BF16, tag="out")
                        nc.scalar.activation(out=out_sb[:], in_=psum2[:],
                                             func=mybir.ActivationFunctionType.Copy,
                                             scale=dinv_t[:, t:t + 1])
                        nc.tensor.matmul(out=pool_acc[:], lhsT=ptall_cm[:, t],
                                         rhs=out_sb[:],
                                         start=(t == 0), stop=(t == tiles - 1))
                    else:
                        # hh = relu(dinv^2 * zW + dinv*b) = dinv * h1
                        nc.scalar.activation(out=hh[:, (t - t0) * D:(t - t0 + 1) * D],
                                             in_=psum2[:],
                                             func=mybir.ActivationFunctionType.Relu,
                                             scale=dinv_t[:, t:t + 1])
                    cola += C + 1
                    if not last_layer and t % GT == GT - 1:
                        nc.sync.dma_start(
                            out=hhat[:, t0 * D:(t0 + GT) * D],
                            in_=hh[:])
                if last_layer and parts != "gather":
                    pooled_sb = mp.tile([P, D], F32, tag="pool_out")
                    nc.scalar.activation(out=pooled_sb[:], in_=pool_acc[:],
                                         func=mybir.ActivationFunctionType.Copy)
                    nc.sync.dma_start(out=pooled[:], in_=pooled_sb[:])
    nc.compile()
    return nc


def _slab_T(slab, tiles):
    # [tiles*P, D] row-major -> [P, tiles*D] per-partition-contiguous
    return np.ascontiguousarray(
        slab.reshape(tiles, P, D).transpose(1, 0, 2).reshape(P, tiles * D))


def _in_maps(pre, table_np, W, b, last_layer):
    maps = []
    rpc = pre['rows_per_core']
    tiles = pre['tiles_per_core']
    t_lo = np.ascontiguousarray(table_np[:SPLIT])
    t_hi = np.ascontiguousarray(table_np[SPLIT:])
    for k in range(N_CORES):
        m = dict(table_lo=t_lo, table_hi=t_hi,
                 self_rows=_slab_T(table_np[k * rpc:(k + 1) * rpc], tiles),
                 idx_lo=pre['idx_lo_sb'][k],
                 idx_hi=pre['idx_hi_sb'][k],
                 dstoff=pre['dstoff'][k],
                 iota_b=pre['iota_jm'],
                 **({'iota_p': pre['iota_pool']} if last_layer else {}),
                 W=np.ascontiguousarray(W, dtype=np.float32),
                 b=np.ascontiguousarray(b, dtype=np.float32).reshape(1, D),
                 dinv=pre['dinv_slab'][k] if last_layer else pre['dinv2_slab'][k],
                 sdeg=pre['sdeg_rows'][k])
        if last_layer:
            m['batchoff'] = pre['batchoff'][k]
        maps.append(m)
    return maps


def kernel(x, edge_index, batch, W1, b1, W2, b2):
    x = np.asarray(x); edge_index = np.asarray(edge_index)
    batch = np.asarray(batch)
    W1 = np.asarray(W1); b1 = np.asarray(b1)
    W2 = np.asarray(W2); b2 = np.asarray(b2)

    pre = preprocess(x, edge_index, batch)
    core_ids = list(range(N_CORES))

    tdt = ml_dtypes.bfloat16 if USE_BF16 else np.float32
    table1 = pre['xhat'].astype(tdt)
    nc1 = build_layer(pre, last_layer=False, bf16_table=USE_BF16)
    res1 = run_bass_kernel_spmd(nc1, _in_maps(pre, table1, W1, b1, False),
                                core_ids).results

    h1hat = np.zeros((pre['npad'], D), dtype=tdt)
    rpc = pre['rows_per_core']
    tiles = pre['tiles_per_core']
    for k in range(N_CORES):
        h1hat[k * rpc:(k + 1) * rpc] = (
            res1[k]['hhat'].reshape(P, tiles, D).transpose(1, 0, 2)
            .reshape(rpc, D))

    nc2 = build_layer(pre, last_layer=True, bf16_table=USE_BF16)
    res2 = run_bass_kernel_spmd(nc2, _in_maps(pre, h1hat, W2, b2, True),
                                core_ids).results

    pooled = np.zeros((G, D), dtype=np.float32)
    for k in range(N_CORES):
        part = res2[k]['pooled']
        g0 = int(pre['g0'][k])
        span = min(P, G - g0)
        pooled[g0:g0 + span] += part[:span]
    return pooled / np.maximum(pre['cnt_g'], 1.0)[:, None]


# revision 66
# speedup vs baseline: 7.8013x; 1.1897x over previous
"""Trainium2 Bass kernel for a 2-layer GCN encoder (GCNConv x2 + global mean pool).

Math: with A' = A + I and deg = indegree(A') (symmetric-norm GCN),
    gcn(h, W, b) = D^-1/2 A' D^-1/2 (h) W + b
factorized as  out = dinv * (A' @ (dinv * h)) @ W + b   (dinv = deg^-1/2)
so the SpMM is pure 0/1 structure; per-edge norms become per-node row scales.

Sharding: dst-node ranges across 8 cores (6272 padded rows each). Each core:
  - bulk-gathers source rows of the (replicated) scaled feature table from HBM
    with InstDMAGatherAnt (one SWDGE op per ~7-tile group instead of one
    indirect op per 128 rows: the ~1us/op descriptor-generation fixed cost
    dominated the old kernel). Indices are int16, so the table is split at row
    32768 into lo/hi halves with one gather each per group,
  - streams each tile's self-loop block with one regular HWDGE DMA per group,
  - builds the 0/1 scatter matrix on VectorE in bf16 (compare dst-offset vs
    iota; the self-loop chunk's column is an identity pattern),
  - scatter-adds via TensorE one-hot matmuls accumulating in PSUM (psum_T is
    [din, dstoff], already transposed for the dense W matmul),
  - dense W matmul + rank-1 bias matmul; ScalarE applies relu and the row
    scale (layer 1 uses scale=dinv^2, folding both dinv factors around the
    relu: dinv*relu(dinv*zW + b) == relu(dinv^2*zW*... ) since dinv > 0).
Two SPMD launches (layer 1 -> host allgather of the 1.6MB/core slabs ->
layer 2 + graph pooling via one-hot matmul over sorted batch ids).
"""
import math
import numpy as np
import ml_dtypes

from concourse import bass, mybir, tile, bacc
from concourse.bass_utils import run_bass_kernel_spmd
from concourse._compat import get_trn_type

N_CORES = 8
P = 128          # partitions / tile rows
D = 128          # feature dim
G = 512          # number of graphs (fixed by the problem)
SPLIT = 32768    # dma_gather int16 index limit: table split row
GT = 7           # dst tiles per self-rows DMA group (49 = 7 x 7)
GATHER_BUDGET = 896   # 56+1 descs/lane: under the 64-desc packet ceiling
SINGLE_PACKET = True  # one packet per lane per op drains far faster
N_SWDGE_Q = 4         # round-robin gathers over SWDGE queues: each SDMA
                      # engine interleaves queue contexts, ~4x gather rate
F32 = mybir.dt.float32
BF16 = mybir.dt.bfloat16
I16 = mybir.dt.int16

USE_BF16 = True     # bf16 gather table (half the random-gather bytes)


def _windows(total_chunks, budget_slots):
    """Slice a chunk stream into fixed-size gather windows (last one ragged).
    Windows are tile-agnostic: a tile's chunks may span two windows."""
    W = budget_slots // P
    return [(w * W, min(W, total_chunks - w * W))
            for w in range((total_chunks + W - 1) // W)]


# ---------------------------------------------------------------- host prep

def preprocess(x, edge_index, batch):
    N = x.shape[0]
    rows_per_core = int(math.ceil(N / (N_CORES * P))) * P
    npad = rows_per_core * N_CORES
    tiles = rows_per_core // P
    n_tiles_g = N_CORES * tiles

    src = edge_index[0].astype(np.int64)
    dst = edge_index[1].astype(np.int64)
    # Self-loops are handled as a per-tile sequential-DMA chunk (no gather
    # slots); only real edges go through the sorted streams.
    deg = (np.bincount(dst, minlength=N) + 1).astype(np.float32)
    dinv = 1.0 / np.sqrt(np.maximum(deg, 1.0))

    xhat = np.zeros((npad, D), dtype=np.float32)
    xhat[:N] = x.astype(np.float32) * dinv[:, None]

    order = np.argsort(dst, kind="stable")
    src_s = src[order].astype(np.int64)
    dst_s = dst[order]
    bounds = np.searchsorted(dst_s, np.arange(0, npad + 1, P))

    # per-global-tile lo/hi edge partition (src < SPLIT goes to the lo gather)
    lo_srcs, lo_offs, hi_srcs, hi_offs = [], [], [], []
    n_lo = np.zeros(n_tiles_g, dtype=np.int64)
    n_hi = np.zeros(n_tiles_g, dtype=np.int64)
    for gt in range(n_tiles_g):
        s, e = int(bounds[gt]), int(bounds[gt + 1])
        seg_src = src_s[s:e]
        seg_off = (dst_s[s:e] - gt * P).astype(np.float32)
        m = seg_src < SPLIT
        lo_srcs.append(seg_src[m]); lo_offs.append(seg_off[m])
        hi_srcs.append(seg_src[~m] - SPLIT); hi_offs.append(seg_off[~m])
        n_lo[gt] = int(m.sum()); n_hi[gt] = int((~m).sum())

    C_lo = np.ceil(n_lo.reshape(N_CORES, tiles).max(axis=0) / P).astype(np.int64)
    C_hi = np.ceil(n_hi.reshape(N_CORES, tiles).max(axis=0) / P).astype(np.int64)
    c_tot = C_lo + C_hi                      # chunks per tile w/o self chunk
    c_max = int(c_tot.max())
    sum_ca = int(c_tot.sum()) + tiles        # dstoff cols incl self chunk

    L_lo = int(C_lo.sum()) * P               # lo gather slots per core
    L_hi = int(C_hi.sum()) * P
    lo_pos = np.concatenate([[0], np.cumsum(C_lo * P)])
    hi_pos = np.concatenate([[0], np.cumsum(C_hi * P)])

    idx_lo = np.zeros((N_CORES, L_lo), dtype=np.int32)
    idx_hi = np.zeros((N_CORES, L_hi), dtype=np.int32)
    dstoff = np.full((N_CORES, P, sum_ca), float(P), dtype=np.float32)
    for k in range(N_CORES):
        cola = 0
        for t in range(tiles):
            gt = k * tiles + t
            Cl, Ch = int(C_lo[t]), int(C_hi[t])
            nl, nh = int(n_lo[gt]), int(n_hi[gt])
            if nl:
                idx_lo[k, lo_pos[t]:lo_pos[t] + nl] = lo_srcs[gt]
                bo = np.full(Cl * P, float(P), dtype=np.float32)
                bo[:nl] = lo_offs[gt]
                dstoff[k, :, cola:cola + Cl] = bo.reshape(Cl, P).T
            cola += Cl
            if nh:
                idx_hi[k, hi_pos[t]:hi_pos[t] + nh] = hi_srcs[gt]
                bo = np.full(Ch * P, float(P), dtype=np.float32)
                bo[:nh] = hi_offs[gt]
                dstoff[k, :, cola:cola + Ch] = bo.reshape(Ch, P).T
            cola += Ch
            # identity column: self-loop chunk scatters partition e -> column e
            # (pad rows contribute zero because xhat is zero there)
            dstoff[k, :, cola] = np.arange(P, dtype=np.float32)
            cola += 1
    assert cola == sum_ca

    def idx_sbuf(a):
        # dma_gather idx layout: element i at partition i%16, col i//16,
        # replicated across all eight 16-partition stripes.
        sb = a.reshape(-1, 16).T
        return np.ascontiguousarray(np.tile(sb, (8, 1)).astype(np.int16))

    idx_lo_sb = np.stack([idx_sbuf(idx_lo[k]) for k in range(N_CORES)])
    idx_hi_sb = np.stack([idx_sbuf(idx_hi[k]) for k in range(N_CORES)])

    dinv_pad = np.zeros(npad, dtype=np.float32)
    dinv_pad[:N] = dinv
    dinv_slab = dinv_pad.reshape(N_CORES, tiles, P).transpose(0, 2, 1).copy()
    dinv2_slab = (dinv_slab * dinv_slab).copy()
    # sqrt(deg) per row (0 on pad rows): the bias is injected into PSUM as
    # sdeg[p]*b[j]; the later per-row scale turns it into the exact bias.
    sdeg_pad = np.zeros(npad, dtype=np.float32)
    sdeg_pad[:N] = np.sqrt(np.maximum(deg, 1.0))
    sdeg_rows = sdeg_pad.reshape(N_CORES, 1, tiles * P).copy()

    batch_pad = np.full(npad, -1, dtype=np.int64)
    batch_pad[:N] = batch.astype(np.int64)
    g0 = np.zeros(N_CORES, dtype=np.int64)
    batchoff = np.full((N_CORES, P, tiles), float(P), dtype=np.float32)
    for k in range(N_CORES):
        b = batch_pad[k * rows_per_core:(k + 1) * rows_per_core]
        real = b >= 0
        assert real.any()
        g0[k] = b[real].min()
        span = int(b[real].max() - g0[k]) + 1
        assert span <= P - 1, f"graph span {span} exceeds pooling tile"
        off = np.full(rows_per_core, float(P), dtype=np.float32)
        off[real] = (b[real] - g0[k]).astype(np.float32)
        batchoff[k] = off.reshape(tiles, P).T

    # j-major iota const: iota_jm[p, j*(c_max+1)+c] = j. The S build runs
    # j-outer/c-inner so every operand's innermost AP dim is unit-stride,
    # which is what unlocks the DVE 2x 16-bit mode.
    iota_jm = np.tile(np.repeat(np.arange(P, dtype=np.float32), c_max + 1),
                      (P, 1)).astype(ml_dtypes.bfloat16)
    # same j-major layout with a tile axis, for the batched pool-selector
    iota_pool = np.tile(np.repeat(np.arange(P, dtype=np.float32), tiles),
                        (P, 1)).astype(ml_dtypes.bfloat16)
    cnt_g = np.bincount(batch.astype(np.int64), minlength=G).astype(np.float32)

    return dict(N=N, npad=npad, rows_per_core=rows_per_core,
                tiles_per_core=tiles, C_lo=C_lo, C_hi=C_hi, c_max=c_max,
                sum_ca=sum_ca, L_lo=L_lo, L_hi=L_hi,
                lo_pos=lo_pos, hi_pos=hi_pos,
                idx_lo_sb=idx_lo_sb, idx_hi_sb=idx_hi_sb,
                dstoff=dstoff.astype(ml_dtypes.bfloat16),
                dinv_slab=dinv_slab, dinv2_slab=dinv2_slab,
                sdeg_rows=sdeg_rows, g0=g0,
                batchoff=batchoff.astype(ml_dtypes.bfloat16),
                iota_jm=iota_jm, iota_pool=iota_pool, xhat=xhat, cnt_g=cnt_g)


# ---------------------------------------------------------------- device

def build_layer(pre, last_layer: bool, reps: int = 1, bf16_table: bool = True,
                parts: str = "full"):
    """One SPMD program: grouped bulk gathers + SpMM + dense matmul per tile.
    last_layer=False: hhat = relu(dinv^2 * (zW) + dinv*b)  [= dinv*h1 table]
    last_layer=True:  h2 = dinv * zW + b, pooled[goff] += sum h2
    """
    tiles = pre['tiles_per_core']
    C_lo, C_hi = pre['C_lo'], pre['C_hi']
    c_max = pre['c_max']
    sum_ca = pre['sum_ca']
    L_lo, L_hi = pre['L_lo'], pre['L_hi']
    lo_pos, hi_pos = pre['lo_pos'], pre['hi_pos']
    npad = pre['npad']
    assert tiles % GT == 0
    sumC_lo, sumC_hi = int(C_lo.sum()), int(C_hi.sum())
    lo_wins = _windows(sumC_lo, GATHER_BUDGET)
    hi_wins = _windows(sumC_hi, GATHER_BUDGET)
    WCH = GATHER_BUDGET // P      # chunks per full window
    lo_cstart = np.concatenate([[0], np.cumsum(C_lo)])  # global lo chunk idx
    hi_cstart = np.concatenate([[0], np.cumsum(C_hi)])

    TDT = BF16 if bf16_table else F32
    nc = bacc.Bacc(get_trn_type() or "TRN2", target_bir_lowering=False,
                   debug=False, num_swdge_queues=N_SWDGE_Q)
    table_lo = nc.dram_tensor("table_lo", [SPLIT, D], TDT, kind="ExternalInput").ap()
    table_hi = nc.dram_tensor("table_hi", [npad - SPLIT, D], TDT,
                              kind="ExternalInput").ap()
    # slab-transposed [p, t*D] layout: per-partition contiguous group slices
    self_rows = nc.dram_tensor("self_rows", [P, tiles * D], TDT,
                               kind="ExternalInput").ap()
    idx_lo = nc.dram_tensor("idx_lo", [P, L_lo // 16], I16, kind="ExternalInput").ap()
    idx_hi = nc.dram_tensor("idx_hi", [P, L_hi // 16], I16, kind="ExternalInput").ap()
    dstoff = nc.dram_tensor("dstoff", [P, sum_ca], BF16, kind="ExternalInput").ap()
    iota_b = nc.dram_tensor("iota_b", [P, (c_max + 1) * D], BF16,
                            kind="ExternalInput").ap()
    if last_layer:
        iota_p = nc.dram_tensor("iota_p", [P, tiles * D], BF16,
                                kind="ExternalInput").ap()
    Wt = nc.dram_tensor("W", [D, D], F32, kind="ExternalInput").ap()
    bt = nc.dram_tensor("b", [1, D], F32, kind="ExternalInput").ap()
    dinv = nc.dram_tensor("dinv", [P, tiles], F32, kind="ExternalInput").ap()
    sdeg = nc.dram_tensor("sdeg", [1, tiles * P], F32, kind="ExternalInput").ap()
    if last_layer:
        batchoff = nc.dram_tensor("batchoff", [P, tiles], BF16,
                                  kind="ExternalInput").ap()
        pooled = nc.dram_tensor("pooled", [P, D], F32, kind="ExternalOutput").ap()
    else:
        hhat = nc.dram_tensor("hhat", [P, tiles * D], TDT, kind="ExternalOutput").ap()



    with tile.TileContext(nc) as tc:
        with tc.tile_pool(name="const", bufs=1) as cp, \
             tc.tile_pool(name="glo", bufs=16) as glp, \
             tc.tile_pool(name="ghi", bufs=16) as ghp, \
             tc.tile_pool(name="gself", bufs=3) as gsp, \
             tc.tile_pool(name="sel", bufs=4) as sp, \
             tc.tile_pool(name="small", bufs=4) as mp, \
             tc.tile_pool(name="hout", bufs=2) as hp, \
             tc.tile_pool(name="ps1", bufs=2, space="PSUM") as pp1, \
             tc.tile_pool(name="ps2", bufs=2, space="PSUM") as pp2, \
             tc.tile_pool(name="ps3", bufs=2, space="PSUM") as pp3:
            ilo_t = cp.tile([P, L_lo // 16], I16)
            ihi_t = cp.tile([P, L_hi // 16], I16)
            dst_t = cp.tile([P, sum_ca], BF16)
            iob_t = cp.tile([P, (c_max + 1) * D], BF16)
            W_t = cp.tile([D, D], F32)
            b_t = cp.tile([1, D], F32)
            sdeg_t = cp.tile([1, tiles * P], F32)
            dinv_t = cp.tile([P, tiles], F32)
            nc.sync.dma_start(out=ilo_t[:], in_=idx_lo[:])
            nc.sync.dma_start(out=ihi_t[:], in_=idx_hi[:])
            nc.sync.dma_start(out=dst_t[:], in_=dstoff[:])
            nc.sync.dma_start(out=iob_t[:], in_=iota_b[:])
            nc.sync.dma_start(out=W_t[:], in_=Wt[:])
            nc.sync.dma_start(out=b_t[:], in_=bt[:])
            nc.sync.dma_start(out=sdeg_t[:], in_=sdeg[:])
            nc.sync.dma_start(out=dinv_t[:], in_=dinv[:])
            if last_layer:
                boff_t = cp.tile([P, tiles], BF16)
                iop_t = cp.tile([P, tiles * D], BF16)
                nc.sync.dma_start(out=boff_t[:], in_=batchoff[:])
                nc.sync.dma_start(out=iop_t[:], in_=iota_p[:])
                # all 49 pool one-hot selectors in one 2x-mode DVE op
                ptall = cp.tile([P, tiles * D], BF16)
                ptall_cm = ptall[:].rearrange("p (j t) -> p t j", t=tiles)
                nc.vector.tensor_tensor(
                    out=ptall[:].rearrange("p (j t) -> p j t", t=tiles),
                    in0=boff_t[:].rearrange("p (t o) -> p o t",
                                            o=1).to_broadcast([P, D, tiles]),
                    in1=iop_t[:].rearrange("p (j t) -> p j t", t=tiles),
                    op=mybir.AluOpType.is_equal)

            dummy_g = None
            if parts == "compute":
                dummy_g = cp.tile([P, WCH * D], TDT)
                nc.vector.memset(dummy_g[:], 0.0)

            for rep in range(reps):
                if last_layer:
                    pool_acc = pp3.tile([P, D], F32, space="PSUM", tag="pacc")
                cola = 0
                gself = hh = None
                gq = 0
                lo_bufs, hi_bufs = {}, {}
                lo_next = hi_next = 0
                for t in range(tiles):
                    # issue gather windows covering this tile's chunks, plus
                    # two tiles of lookahead so gathers run ahead of compute
                    ta = min(t + 8, tiles)
                    while (lo_next < len(lo_wins)
                           and lo_wins[lo_next][0] < lo_cstart[ta]):
                        c0, nch = lo_wins[lo_next]
                        buf = (dummy_g if parts == "compute"
                               else glp.tile([P, WCH * D], TDT, tag="glo"))
                        if parts != "compute":
                            nc.gpsimd.dma_gather(
                                buf[:, :nch * D].rearrange("p (c j) -> p c j", j=D),
                                table_lo[:],
                                ilo_t[:, c0 * P // 16:(c0 + nch) * P // 16],
                                nch * P, nch * P, D, single_packet=SINGLE_PACKET,
                                queue_num=gq % N_SWDGE_Q)
                        gq += 1
                        lo_bufs[lo_next] = buf
                        lo_next += 1
                    while (hi_next < len(hi_wins)
                           and hi_wins[hi_next][0] < hi_cstart[ta]):
                        c0, nch = hi_wins[hi_next]
                        buf = (dummy_g if parts == "compute"
                               else ghp.tile([P, WCH * D], TDT, tag="ghi"))
                        if parts != "compute":
                            nc.gpsimd.dma_gather(
                                buf[:, :nch * D].rearrange("p (c j) -> p c j", j=D),
                                table_hi[:],
                                ihi_t[:, c0 * P // 16:(c0 + nch) * P // 16],
                                nch * P, nch * P, D, single_packet=SINGLE_PACKET,
                                queue_num=gq % N_SWDGE_Q)
                        gq += 1
                        hi_bufs[hi_next] = buf
                        hi_next += 1
                    if parts == "gather":
                        cola += int(C_lo[t]) + int(C_hi[t]) + 1
                        continue
                    if t % GT == 0:
                        t0 = t
                        gself = gsp.tile([P, GT * D], TDT, tag="gs")
                        nc.sync.dma_start(
                            out=gself[:],
                            in_=self_rows[:, t0 * D:(t0 + GT) * D])
                        if not last_layer:
                            hh = hp.tile([P, GT * D], TDT, tag="hh")
                    Cl, Ch = int(C_lo[t]), int(C_hi[t])
                    C = Cl + Ch
                    # S built j-major (S[p, j, c]) so every operand's innermost
                    # AP dim is unit-stride -> DVE 2x 16-bit mode. The matmul
                    # then reads chunk c as a stride-(C+1) rhs slice.
                    S = sp.tile([P, (c_max + 1) * D], BF16, tag="s")
                    nc.vector.tensor_tensor(
                        out=S[:, :(C + 1) * D].rearrange("p (j c) -> p j c",
                                                         c=C + 1),
                        in0=dst_t[:, cola:cola + C + 1].rearrange(
                            "p (c o) -> p o c", o=1).to_broadcast([P, D, C + 1]),
                        in1=iob_t[:].rearrange("p (j c) -> p j c",
                                               c=c_max + 1)[:, :, :C + 1],
                        op=mybir.AluOpType.is_equal)
                    S_cm = S[:, :(C + 1) * D].rearrange("p (j c) -> p c j",
                                                        c=C + 1)
                    psumT = pp1.tile([P, D], F32, space="PSUM", tag="pT")
                    for c in range(C + 1):
                        if c < Cl:
                            q = int(lo_cstart[t]) + c
                            lhsT = lo_bufs[q // WCH][:, (q % WCH) * D:
                                                     (q % WCH + 1) * D]
                        elif c < C:
                            q = int(hi_cstart[t]) + (c - Cl)
                            lhsT = hi_bufs[q // WCH][:, (q % WCH) * D:
                                                     (q % WCH + 1) * D]
                        else:
                            lhsT = gself[:, (t - t0) * D:(t - t0 + 1) * D]
                        nc.tensor.matmul(out=psumT[:], lhsT=lhsT,
                                         rhs=S_cm[:, c],
                                         start=(c == 0), stop=(c == C))
                    lhs_sb = mp.tile([P, D], F32, tag="lhs")
                    nc.scalar.activation(out=lhs_sb[:], in_=psumT[:],
                                         func=mybir.ActivationFunctionType.Copy)
                    psum2 = pp2.tile([P, D], F32, space="PSUM", tag="p2")
                    nc.tensor.matmul(out=psum2[:], lhsT=lhs_sb[:], rhs=W_t[:],
                                     start=True, stop=False)
                    nc.tensor.matmul(out=psum2[:],
                                     lhsT=sdeg_t[:, t * P:(t + 1) * P],
                                     rhs=b_t[:], start=False, stop=True)
                    if last_layer:
                        out_sb = mp.tile([P, D], BF16, tag="out")
                        nc.scalar.activation(out=out_sb[:], in_=psum2[:],
                                             func=mybir.ActivationFunctionType.Copy,
                                             scale=dinv_t[:, t:t + 1])
                        nc.tensor.matmul(out=pool_acc[:], lhsT=ptall_cm[:, t],
                                         rhs=out_sb[:],
                                         start=(t == 0), stop=(t == tiles - 1))
                    else:
                        # hh = relu(dinv^2 * zW + dinv*b) = dinv * h1
                        nc.scalar.activation(out=hh[:, (t - t0) * D:(t - t0 + 1) * D],
                                             in_=psum2[:],
                                             func=mybir.ActivationFunctionType.Relu,
                                             scale=dinv_t[:, t:t + 1])
                    cola += C + 1
                    if not last_layer and t % GT == GT - 1:
                        nc.sync.dma_start(
                            out=hhat[:, t0 * D:(t0 + GT) * D],
                            in_=hh[:])
                if last_layer and parts != "gather":
                    pooled_sb = mp.tile([P, D], F32, tag="pool_out")
                    nc.scalar.activation(out=pooled_sb[:], in_=pool_acc[:],
                                         func=mybir.ActivationFunctionType.Copy)
                    nc.sync.dma_start(out=pooled[:], in_=pooled_sb[:])
    nc.compile()
    return nc


def _slab_T(slab, tiles):
    # [tiles*P, D] row-major -> [P, tiles*D] per-partition-contiguous
    return np.ascontiguousarray(
        slab.reshape(tiles, P, D).transpose(1, 0, 2).reshape(P, tiles * D))


def _in_maps(pre, table_np, W, b, last_layer):
    maps = []
    rpc = pre['rows_per_core']
    tiles = pre['tiles_per_core']
    t_lo = np.ascontiguousarray(table_np[:SPLIT])
    t_hi = np.ascontiguousarray(table_np[SPLIT:])
    for k in range(N_CORES):
        m = dict(table_lo=t_lo, table_hi=t_hi,
                 self_rows=_slab_T(table_np[k * rpc:(k + 1) * rpc], tiles),
                 idx_lo=pre['idx_lo_sb'][k],
                 idx_hi=pre['idx_hi_sb'][k],
                 dstoff=pre['dstoff'][k],
                 iota_b=pre['iota_jm'],
                 **({'iota_p': pre['iota_pool']} if last_layer else {}),
                 W=np.ascontiguousarray(W, dtype=np.float32),
                 b=np.ascontiguousarray(b, dtype=np.float32).reshape(1, D),
                 dinv=pre['dinv_slab'][k] if last_layer else pre['dinv2_slab'][k],
                 sdeg=pre['sdeg_rows'][k])
        if last_layer:
            m['batchoff'] = pre['batchoff'][k]
        maps.append(m)
    return maps


def kernel(x, edge_index, batch, W1, b1, W2, b2):
    x = np.asarray(x); edge_index = np.asarray(edge_index)
    batch = np.asarray(batch)
    W1 = np.asarray(W1); b1 = np.asarray(b1)
    W2 = np.asarray(W2); b2 = np.asarray(b2)

    pre = preprocess(x, edge_index, batch)
    core_ids = list(range(N_CORES))

    tdt = ml_dtypes.bfloat16 if USE_BF16 else np.float32
    table1 = pre['xhat'].astype(tdt)
    nc1 = build_layer(pre, last_layer=False, bf16_table=USE_BF16)
    res1 = run_bass_kernel_spmd(nc1, _in_maps(pre, table1, W1, b1, False),
                                core_ids).results

    h1hat = np.zeros((pre['npad'], D), dtype=tdt)
    rpc = pre['rows_per_core']
    tiles = pre['tiles_per_core']
    for k in range(N_CORES):
        h1hat[k * rpc:(k + 1) * rpc] = (
            res1[k]['hhat'].reshape(P, tiles, D).transpose(1, 0, 2)
            .reshape(rpc, D))

    nc2 = build_layer(pre, last_layer=True, bf16_table=USE_BF16)
    res2 = run_bass_kernel_spmd(nc2, _in_maps(pre, h1hat, W2, b2, True),
                                core_ids).results

    pooled = np.zeros((G, D), dtype=np.float32)
    for k in range(N_CORES):
        part = res2[k]['pooled']
        g0 = int(pre['g0'][k])
        span = min(P, G - g0)
        pooled[g0:g0 + span] += part[:span]
    return pooled / np.maximum(pre['cnt_g'], 1.0)[:, None]
